# revision 1
# baseline (speedup 1.0000x reference)
"""Bass/Trainium2 SPMD kernel for nn_JittableSelfAttention_Rels.

The reference's softmax is over the singleton query dim, so attention
weights are identically 1 and the output reduces to

    out[1,128] = sum_{e: mask[e]} ( v_[neighbors[e]]
                                    + t2v(times[e]) @ W_tv
                                    + rels[e] @ W_rv )

where W_tv / W_rv are the v-thirds of time_kqv_w / edge_kqv_w.

Sharding: v_ is split row-wise across 8 cores. The host routes each edge
to the core owning its neighbor row (weight = mask AND owned, else 0);
every core runs the same program over all 2048 edge slots, gathers its
shard's rows with one indirect DMA, and produces a [128] partial sum.
The host adds the 8 partials.
"""

import sys

import numpy as np

if "/opt/trn_rl_repo" not in sys.path:
    sys.path.insert(0, "/opt/trn_rl_repo")

N_NODES = 1_000_000
E = 2048
HIDDEN = 128
P = 128
NCH = E // P  # 16 chunks of 128 edges
NCORES = 8
ROWS = N_NODES // NCORES  # 125000
T_DIM = 64
R_DIM = 32

_CACHE = {}


def _build_program():
    import concourse.bass as bass
    import concourse.tile as tile
    from concourse import bacc, mybir

    f32 = mybir.dt.float32
    i32 = mybir.dt.int32

    nc = bacc.Bacc()
    v_shard = nc.declare_dram_parameter("v_shard", [ROWS, HIDDEN], f32, isOutput=False)
    idx_pc = nc.declare_dram_parameter("idx_pc", [P, NCH], i32, isOutput=False)
    wts_pc = nc.declare_dram_parameter("wts_pc", [P, NCH], f32, isOutput=False)
    tm_pc = nc.declare_dram_parameter("tm_pc", [P, NCH], f32, isOutput=False)
    rels_pc = nc.declare_dram_parameter(
        "rels_pc", [P, NCH, R_DIM], f32, isOutput=False
    )
    wf_bc = nc.declare_dram_parameter("wf_bc", [P, T_DIM], f32, isOutput=False)
    bf_bc = nc.declare_dram_parameter("bf_bc", [P, T_DIM], f32, isOutput=False)
    w_tv = nc.declare_dram_parameter("w_tv", [T_DIM, HIDDEN], f32, isOutput=False)
    w_rv = nc.declare_dram_parameter("w_rv", [R_DIM, HIDDEN], f32, isOutput=False)
    out = nc.declare_dram_parameter("out", [HIDDEN], f32, isOutput=True)

    with tile.TileContext(nc) as tc:
        with (
            tc.tile_pool(name="sb", bufs=1) as pool,
            tc.tile_pool(name="ps", bufs=1, space="PSUM") as psum,
        ):
            idx_t = pool.tile([P, NCH], i32)
            wts_t = pool.tile([P, NCH], f32)
            tm_t = pool.tile([P, NCH], f32)
            rels_t = pool.tile([P, NCH * R_DIM], f32)
            wf_t = pool.tile([P, T_DIM], f32)
            bf_t = pool.tile([P, T_DIM], f32)
            wtv_t = pool.tile([T_DIM, HIDDEN], f32)
            wrv_t = pool.tile([R_DIM, HIDDEN], f32)
            gath_t = pool.tile([P, NCH * HIDDEN], f32)
            arg_t = pool.tile([P, NCH * T_DIM], f32)
            wrp_t = pool.tile([P, NCH * T_DIM], f32)
            ge_t = pool.tile([P, NCH * T_DIM], f32)
            lt_t = pool.tile([P, NCH * T_DIM], f32)
            te_t = pool.tile([P, NCH * T_DIM], f32)
            tes_t = pool.tile([T_DIM, 1], f32)
            rs_t = pool.tile([R_DIM, 1], f32)
            outc_t = pool.tile([P, 1], f32)
            pi_t = pool.tile([P, 1], f32)
            negpi_t = pool.tile([P, 1], f32)

            nc.sync.dma_start(out=idx_t[:], in_=idx_pc[:])
            nc.sync.dma_start(out=wts_t[:], in_=wts_pc[:])
            nc.sync.dma_start(out=tm_t[:], in_=tm_pc[:])
            nc.sync.dma_start(
                out=rels_t[:], in_=rels_pc[:].rearrange("p c r -> p (c r)")
            )
            nc.sync.dma_start(out=wf_t[:], in_=wf_bc[:])
            nc.sync.dma_start(out=bf_t[:], in_=bf_bc[:])
            nc.sync.dma_start(out=wtv_t[:], in_=w_tv[:])
            nc.sync.dma_start(out=wrv_t[:], in_=w_rv[:])

            # Gather v rows: edge (p, c) = p*NCH + c lands in
            # gath_t[p, c*HIDDEN:(c+1)*HIDDEN]. One indirect DMA per chunk:
            # HW consumes exactly one row index per partition (sim's
            # multi-column offset ravel does NOT match hardware).
            for c in range(NCH):
                nc.gpsimd.indirect_dma_start(
                    out=gath_t[:, c * HIDDEN : (c + 1) * HIDDEN],
                    out_offset=None,
                    in_=v_shard[:],
                    in_offset=bass.IndirectOffsetOnAxis(
                        ap=idx_t[:, c : c + 1], axis=0
                    ),
                )

            # Time2Vec over all edges: arg = t*w + b with edge on partitions,
            # chunk and time-dim on the free axis.
            tm_b = (
                tm_t[:]
                .rearrange("p (c o) -> p c o", o=1)
                .to_broadcast([P, NCH, T_DIM])
            )
            wf_b = (
                wf_t[:]
                .rearrange("p (o j) -> p o j", o=1)
                .to_broadcast([P, NCH, T_DIM])
            )
            bf_b = (
                bf_t[:]
                .rearrange("p (o j) -> p o j", o=1)
                .to_broadcast([P, NCH, T_DIM])
            )
            arg3 = arg_t[:].rearrange("p (c j) -> p c j", j=T_DIM)
            wrp3 = wrp_t[:].rearrange("p (c j) -> p c j", j=T_DIM)
            te3 = te_t[:].rearrange("p (c j) -> p c j", j=T_DIM)
            nc.vector.tensor_tensor(
                out=arg3, in0=tm_b, in1=wf_b, op=mybir.AluOpType.mult
            )
            nc.vector.tensor_tensor(
                out=arg3, in0=arg3, in1=bf_b, op=mybir.AluOpType.add
            )
            # Scalar-engine Sin only accepts [-pi, pi]. Arguments here are
            # t*w + b with t in [0,1), w,b ~ N(0,1), so a single-step wrap
            # (covers |arg| <= 3*pi) suffices:
            #   wrapped = arg - 2*pi * (is_gt(arg, pi) - is_lt(arg, -pi))
            pi = float(np.pi)
            nc.vector.memset(pi_t[:], pi)
            nc.vector.memset(negpi_t[:], -pi)
            pi_b = (
                pi_t[:]
                .rearrange("p (a b) -> p a b", a=1)
                .to_broadcast([P, NCH, T_DIM])
            )
            negpi_b = (
                negpi_t[:]
                .rearrange("p (a b) -> p a b", a=1)
                .to_broadcast([P, NCH, T_DIM])
            )
            ge3 = ge_t[:].rearrange("p (c j) -> p c j", j=T_DIM)
            lt3 = lt_t[:].rearrange("p (c j) -> p c j", j=T_DIM)
            nc.vector.tensor_tensor(
                out=ge3, in0=arg3, in1=pi_b, op=mybir.AluOpType.is_gt
            )
            nc.vector.tensor_tensor(
                out=lt3, in0=arg3, in1=negpi_b, op=mybir.AluOpType.is_lt
            )
            nc.vector.tensor_tensor(
                out=ge3, in0=ge3, in1=lt3, op=mybir.AluOpType.subtract
            )
            nc.vector.tensor_scalar_mul(ge3, ge3, 2.0 * pi)
            nc.vector.tensor_tensor(
                out=wrp3, in0=arg3, in1=ge3, op=mybir.AluOpType.subtract
            )
            # Channel 0 is the linear Time2Vec channel; 1..63 are sin().
            nc.scalar.activation(
                out=te3[:, :, 1:],
                in_=wrp3[:, :, 1:],
                func=mybir.ActivationFunctionType.Sin,
            )
            nc.vector.tensor_copy(out=te3[:, :, 0:1], in_=arg3[:, :, 0:1])

            ps_te = psum.tile([T_DIM, 1], f32, space="PSUM")
            ps_r = psum.tile([R_DIM, 1], f32, space="PSUM")
            ps_col = psum.tile([P, 1], f32, space="PSUM")

            # Weighted sums over edges (contract the 128-partition dim),
            # producing column vectors: te_sum [64,1], r_sum [32,1].
            for c in range(NCH):
                nc.tensor.matmul(
                    out=ps_te[:],
                    lhsT=te_t[:, c * T_DIM : (c + 1) * T_DIM],
                    rhs=wts_t[:, c : c + 1],
                    start=(c == 0),
                    stop=(c == NCH - 1),
                )
            for c in range(NCH):
                nc.tensor.matmul(
                    out=ps_r[:],
                    lhsT=rels_t[:, c * R_DIM : (c + 1) * R_DIM],
                    rhs=wts_t[:, c : c + 1],
                    start=(c == 0),
                    stop=(c == NCH - 1),
                )
            nc.vector.tensor_copy(out=tes_t[:], in_=ps_te[:])
            nc.vector.tensor_copy(out=rs_t[:], in_=ps_r[:])

            # v-sum as a column [128,1], then add the two projections
            # W_tv^T te_sum and W_rv^T r_sum into the same accumulator.
            for c in range(NCH):
                nc.tensor.matmul(
                    out=ps_col[:],
                    lhsT=gath_t[:, c * HIDDEN : (c + 1) * HIDDEN],
                    rhs=wts_t[:, c : c + 1],
                    start=(c == 0),
                    stop=False,
                )
            nc.tensor.matmul(
                out=ps_col[:], lhsT=wtv_t[:], rhs=tes_t[:], start=False, stop=False
            )
            nc.tensor.matmul(
                out=ps_col[:], lhsT=wrv_t[:], rhs=rs_t[:], start=False, stop=True
            )
            nc.vector.tensor_copy(out=outc_t[:], in_=ps_col[:])
            nc.sync.dma_start(
                out=out[:].rearrange("(p o) -> p o", o=1), in_=outc_t[:]
            )

    nc.compile()
    return nc


def _prep_in_maps(v_, neighbors, mask, times, rels, w0, b0, w, b, Wt, We):
    v_np = np.ascontiguousarray(np.asarray(v_, dtype=np.float32))
    nb = np.asarray(neighbors).astype(np.int64).ravel()
    m = np.asarray(mask).astype(bool).ravel()
    t = np.ascontiguousarray(np.asarray(times, dtype=np.float32).ravel())
    rels_np = np.ascontiguousarray(np.asarray(rels, dtype=np.float32))

    wf = np.empty(T_DIM, np.float32)
    bf = np.empty(T_DIM, np.float32)
    wf[0] = np.float32(np.asarray(w0))
    wf[1:] = np.asarray(w, dtype=np.float32).ravel()
    bf[0] = np.float32(np.asarray(b0))
    bf[1:] = np.asarray(b, dtype=np.float32).ravel()
    wf_bc = np.ascontiguousarray(np.broadcast_to(wf, (P, T_DIM)))
    bf_bc = np.ascontiguousarray(np.broadcast_to(bf, (P, T_DIM)))

    Wtv = np.ascontiguousarray(
        np.asarray(Wt, dtype=np.float32)[:, 2 * HIDDEN : 3 * HIDDEN]
    )
    Wrv = np.ascontiguousarray(
        np.asarray(We, dtype=np.float32)[:, 2 * HIDDEN : 3 * HIDDEN]
    )

    tm_pc = t.reshape(P, NCH)
    rels_pc = rels_np.reshape(P, NCH, R_DIM)

    owner = nb // ROWS
    in_maps = []
    for c in range(NCORES):
        own = owner == c
        wts = (m & own).astype(np.float32).reshape(P, NCH)
        lidx = (
            np.where(own, np.clip(nb - c * ROWS, 0, ROWS - 1), 0)
            .astype(np.int32)
            .reshape(P, NCH)
        )
        in_maps.append(
            {
                "v_shard": v_np[c * ROWS : (c + 1) * ROWS],
                "idx_pc": np.ascontiguousarray(lidx),
                "wts_pc": np.ascontiguousarray(wts),
                "tm_pc": tm_pc,
                "rels_pc": rels_pc,
                "wf_bc": wf_bc,
                "bf_bc": bf_bc,
                "w_tv": Wtv,
                "w_rv": Wrv,
            }
        )
    return in_maps


def kernel(
    k_,
    q_,
    v_,
    neighbors,
    nid,
    mask,
    start_t,
    times,
    rels,
    t2v_w0,
    t2v_b0,
    t2v_w,
    t2v_b,
    time_kqv_w,
    edge_kqv_w,
):
    from concourse.bass_utils import run_bass_kernel_spmd

    nc = _CACHE.get("nc")
    if nc is None:
        nc = _build_program()
        _CACHE["nc"] = nc

    in_maps = _prep_in_maps(
        v_, neighbors, mask, times, rels, t2v_w0, t2v_b0, t2v_w, t2v_b,
        time_kqv_w, edge_kqv_w,
    )
    res = run_bass_kernel_spmd(nc, in_maps, list(range(NCORES)))
    partials = np.stack([r["out"] for r in res.results]).astype(np.float32)
    return partials.sum(axis=0, dtype=np.float32).reshape(1, HIDDEN)



# revision 11
# speedup vs baseline: 2.0493x; 2.0493x over previous
"""Bass/Trainium2 SPMD kernel for nn_JittableSelfAttention_Rels.

The reference's softmax is over the singleton query dim, so attention
weights are identically 1 and the output reduces to

    out[1,128] = sum_{e: mask[e]} ( v_[neighbors[e]]
                                    + t2v(times[e]) @ W_tv
                                    + rels[e] @ W_rv )

where W_tv / W_rv are the v-thirds of time_kqv_w / edge_kqv_w.

Sharding: v_ is split row-wise across 8 cores. The host routes each
masked edge to the core owning its neighbor row (<=256 slots per core,
11-sigma headroom; overflow edges are summed on the host, which in
practice never triggers). Each core gathers its routed rows with two
128-row indirect DMAs and reduces them with transposed matmuls into a
single-partition [1,128] PSUM row (single-partition output keeps the
final HBM write on one DMA engine, avoiding the 16-way semaphore
trickle). The Time2Vec/rels contributions are position-sharded: core c
handles edges [256c, 256c+256) with T_DIM on partitions, so the whole
sin argument is one fused tensor_scalar, the +-pi range reduction is a
single mod op (arguments pre-biased by +9*pi on the host so trunc- and
floor-mod agree), and the masked sums are fused multiply+reduce ops.
The host adds the 8 partial [128] outputs.
"""

import sys

import numpy as np

if "/opt/trn_rl_repo" not in sys.path:
    sys.path.insert(0, "/opt/trn_rl_repo")

N_NODES = 1_000_000
E = 2048
HIDDEN = 128
P = 128
NCORES = 8
ROWS = N_NODES // NCORES  # 125000
T_DIM = 64
R_DIM = 32
EB = E // NCORES  # 256 edges per core for the t2v/rels block
NCHV = 2          # gather chunks per core
CAP = NCHV * P    # 256 routed v-row slots per core

# packed-parameter column layout ([128, F_COLS] f32)
# The linear Time2Vec channel rides as row 32 of the rels block (its
# values w0*t+b0 are a host-side affine of the shipped times), so every
# engine op can use partition ranges starting at 0 (the BIR verifier
# rejects odd partition offsets like [1:64]).
C_TM = 0                  # rows 0..63:  times block, replicated x64
C_EW = C_TM + EB          # rows 0..63:  mask block (f32), replicated x64
C_RT = C_EW + EB          # rows 0..31:  rels block, transposed [32, 256];
                          #              row 32: linear-channel values;
                          #              rows 33..63: zero
C_W = C_RT + EB           # rows 0..63:  col C_W = w vec, col C_W+1 = b vec,
                          #              col C_W+2 unused
C_WTV = C_W + 3           # rows 0..63:  W_tv [64, 128] with row 0 zeroed
C_WRV = C_WTV + HIDDEN    # rows 0..31:  W_rv; row 32: W_tv's true row 0
C_VW = C_WRV + HIDDEN     # rows 0..127: routed-slot weights [128, NCHV]
F_COLS = C_VW + NCHV

_CACHE = {}


def _build_program():
    import concourse.bass as bass
    import concourse.tile as tile
    from concourse import bacc, mybir

    f32 = mybir.dt.float32
    i32 = mybir.dt.int32
    PI = float(np.pi)
    ALU = mybir.AluOpType

    nc = bacc.Bacc()
    v_shard = nc.declare_dram_parameter("v_shard", [ROWS, HIDDEN], f32, isOutput=False)
    idx_pc = nc.declare_dram_parameter("idx_pc", [P, NCHV], i32, isOutput=False)
    pk = nc.declare_dram_parameter("pk", [P, F_COLS], f32, isOutput=False)
    out = nc.declare_dram_parameter("out", [HIDDEN], f32, isOutput=True)

    with tile.TileContext(nc) as tc:
        with (
            tc.tile_pool(name="sb", bufs=1) as pool,
            tc.tile_pool(name="ps", bufs=1, space="PSUM") as psum,
        ):
            idx_t = pool.tile([P, NCHV], i32)
            pk_t = pool.tile([P, F_COLS], f32)
            gath = [
                pool.tile([P, HIDDEN], f32, tag=f"g{j}", name=f"gath{j}")
                for j in range(NCHV)
            ]
            te_t = pool.tile([T_DIM, EB], f32)   # arg, then sin(arg)
            u_t = pool.tile([T_DIM, EB], f32)    # arg/2pi + magic
            qm_t = pool.tile([T_DIM, EB], f32)   # -2pi * round(arg/2pi)
            r_t = pool.tile([T_DIM, EB], f32)    # range-reduced argument
            wte_t = pool.tile([T_DIM, EB], f32)  # scratch for fused mul+reduce
            wre_t = pool.tile([T_DIM, EB], f32)
            te_sum = pool.tile([T_DIM, 1], f32)
            r_sum = pool.tile([T_DIM, 1], f32)
            ocol_t = pool.tile([P, 1], f32)
            ps_col = psum.tile([P, 1], f32, space="PSUM")

            # Index load first so the gathers start as early as possible.
            nc.sync.dma_start(out=idx_t[:], in_=idx_pc[:])
            for j in range(NCHV):
                nc.gpsimd.indirect_dma_start(
                    out=gath[j][:],
                    out_offset=None,
                    in_=v_shard[:],
                    in_offset=bass.IndirectOffsetOnAxis(
                        ap=idx_t[:, j : j + 1], axis=0
                    ),
                )
            nc.sync.dma_start(out=pk_t[:], in_=pk[:])

            tm_bc = pk_t[0:T_DIM, C_TM : C_TM + EB]
            ew64 = pk_t[0:T_DIM, C_EW : C_EW + EB]
            rels_T = pk_t[0:T_DIM, C_RT : C_RT + EB]
            w_col = pk_t[0:T_DIM, C_W : C_W + 1]
            b_col = pk_t[0:T_DIM, C_W + 1 : C_W + 2]
            wtv = pk_t[0:T_DIM, C_WTV : C_WTV + HIDDEN]
            wrv = pk_t[0:T_DIM, C_WRV : C_WRV + HIDDEN]
            vw = pk_t[:, C_VW : C_VW + NCHV]

            # arg[j,e] = t[e]*w[j] + b[j]; range-reduce to [-pi, pi] for
            # the Sin table via the fp32 magic-number round:
            # r = arg - 2pi * round(arg/2pi). Row 0's sin is junk (the true
            # linear channel rides in the rels block; W_tv row 0 is zeroed).
            MAGIC = 12582912.0  # 1.5 * 2**23
            nc.vector.tensor_scalar(
                out=te_t[:], in0=tm_bc, scalar1=w_col, scalar2=b_col,
                op0=ALU.mult, op1=ALU.add,
            )
            nc.vector.tensor_scalar(
                out=u_t[:], in0=te_t[:], scalar1=1.0 / (2.0 * PI),
                scalar2=MAGIC, op0=ALU.mult, op1=ALU.add,
            )
            nc.vector.tensor_scalar(
                out=qm_t[:], in0=u_t[:], scalar1=MAGIC,
                scalar2=-2.0 * PI, op0=ALU.subtract, op1=ALU.mult,
            )
            nc.vector.tensor_tensor(
                out=r_t[:], in0=te_t[:], in1=qm_t[:], op=ALU.add
            )
            nc.scalar.activation(
                out=te_t[:], in_=r_t[:],
                func=mybir.ActivationFunctionType.Sin,
            )
            # Masked sums over the free (edge) axis.
            nc.vector.tensor_tensor(
                out=wte_t[:], in0=te_t[:], in1=ew64, op=ALU.mult
            )
            nc.vector.tensor_reduce(
                out=te_sum[:], in_=wte_t[:], axis=mybir.AxisListType.X,
                op=ALU.add,
            )
            nc.vector.tensor_tensor(
                out=wre_t[:], in0=rels_T, in1=ew64, op=ALU.mult
            )
            nc.vector.tensor_reduce(
                out=r_sum[:], in_=wre_t[:], axis=mybir.AxisListType.X,
                op=ALU.add,
            )

            # All contributions accumulate into one PSUM column
            # out[h] = sum_p lhsT[p,h] * rhs[p,0].
            for j in range(NCHV):
                nc.tensor.matmul(
                    out=ps_col[:], lhsT=gath[j][:], rhs=vw[:, j : j + 1],
                    start=(j == 0), stop=False,
                )
            nc.tensor.matmul(
                out=ps_col[:], lhsT=wtv, rhs=te_sum[:], start=False, stop=False
            )
            nc.tensor.matmul(
                out=ps_col[:], lhsT=wrv, rhs=r_sum[:], start=False, stop=True
            )
            nc.vector.tensor_copy(out=ocol_t[:], in_=ps_col[:])
            nc.sync.dma_start(
                out=out[:].rearrange("(p o) -> p o", o=1), in_=ocol_t[:]
            )

    nc.compile()
    return nc


def _prep_in_maps(v_, neighbors, mask, times, rels, w0, b0, w, b, Wt, We):
    PI = float(np.pi)
    v_np = np.ascontiguousarray(np.asarray(v_, dtype=np.float32))
    nb = np.asarray(neighbors).astype(np.int64).ravel()
    m = np.asarray(mask).astype(bool).ravel()
    t = np.asarray(times, dtype=np.float32).ravel()
    rels_np = np.asarray(rels, dtype=np.float32)

    Wtv = np.ascontiguousarray(
        np.asarray(Wt, dtype=np.float32)[:, 2 * HIDDEN : 3 * HIDDEN]
    )
    Wrv = np.ascontiguousarray(
        np.asarray(We, dtype=np.float32)[:, 2 * HIDDEN : 3 * HIDDEN]
    )
    w0f = np.float32(np.asarray(w0))
    b0f = np.float32(np.asarray(b0))
    wf = np.empty(T_DIM, np.float32)
    wf[0] = w0f  # row 0 is a junk sin channel (W_tv row 0 zeroed)
    wf[1:] = np.asarray(w, dtype=np.float32).ravel()
    bf = np.empty(T_DIM, np.float32)
    bf[0] = b0f
    bf[1:] = np.asarray(b, dtype=np.float32).ravel()

    owner = nb // ROWS
    local = (nb - owner * ROWS).astype(np.int32)

    in_maps = []
    spill = np.zeros(HIDDEN, np.float64)
    for c in range(NCORES):
        sel = np.nonzero(m & (owner == c))[0]
        if len(sel) > CAP:  # ~11 sigma; handled on host for correctness
            over = sel[CAP:]
            sel = sel[:CAP]
            spill += v_np[nb[over]].sum(axis=0, dtype=np.float64)
        n = len(sel)
        idx = np.zeros(CAP, np.int32)
        wts = np.zeros(CAP, np.float32)
        idx[:n] = local[sel]
        wts[:n] = 1.0
        # slot s <-> (chunk j = s // P, partition p = s % P)
        idx_pc = np.ascontiguousarray(idx.reshape(NCHV, P).T)
        wts_pc = np.ascontiguousarray(wts.reshape(NCHV, P).T)

        blk = slice(EB * c, EB * (c + 1))
        pk = np.zeros((P, F_COLS), np.float32)
        pk[0:T_DIM, C_TM : C_TM + EB] = t[blk][None, :]
        pk[0:T_DIM, C_EW : C_EW + EB] = m[blk].astype(np.float32)[None, :]
        pk[0:R_DIM, C_RT : C_RT + EB] = rels_np[blk].T
        pk[R_DIM, C_RT : C_RT + EB] = w0f * t[blk] + b0f  # linear channel
        pk[0:T_DIM, C_W] = wf
        pk[0:T_DIM, C_W + 1] = bf
        pk[0:T_DIM, C_WTV : C_WTV + HIDDEN] = Wtv
        pk[0, C_WTV : C_WTV + HIDDEN] = 0.0  # linear channel moved to rels row
        pk[0:R_DIM, C_WRV : C_WRV + HIDDEN] = Wrv
        pk[R_DIM, C_WRV : C_WRV + HIDDEN] = Wtv[0]
        pk[:, C_VW : C_VW + NCHV] = wts_pc

        in_maps.append(
            {
                "v_shard": v_np[c * ROWS : (c + 1) * ROWS],
                "idx_pc": idx_pc,
                "pk": pk,
            }
        )
    return in_maps, spill


def kernel(
    k_,
    q_,
    v_,
    neighbors,
    nid,
    mask,
    start_t,
    times,
    rels,
    t2v_w0,
    t2v_b0,
    t2v_w,
    t2v_b,
    time_kqv_w,
    edge_kqv_w,
):
    from concourse.bass_utils import run_bass_kernel_spmd

    nc = _CACHE.get("nc")
    if nc is None:
        nc = _build_program()
        _CACHE["nc"] = nc

    in_maps, spill = _prep_in_maps(
        v_, neighbors, mask, times, rels, t2v_w0, t2v_b0, t2v_w, t2v_b,
        time_kqv_w, edge_kqv_w,
    )
    res = run_bass_kernel_spmd(nc, in_maps, list(range(NCORES)))
    total = np.sum(
        np.stack([r["out"] for r in res.results]).astype(np.float64), axis=0
    )
    return (total + spill).astype(np.float32).reshape(1, HIDDEN)


# revision 13
# speedup vs baseline: 2.3014x; 1.1230x over previous
"""Bass/Trainium2 SPMD kernel for nn_JittableSelfAttention_Rels.

The reference's softmax is over the singleton query dim, so attention
weights are identically 1 and the output reduces to

    out[1,128] = sum_{e: mask[e]} ( v_[neighbors[e]]
                                    + t2v(times[e]) @ W_tv
                                    + rels[e] @ W_rv )

where W_tv / W_rv are the v-thirds of time_kqv_w / edge_kqv_w.

Sharding: v_ is split row-wise across 8 cores. The host routes each
masked edge to the core owning its neighbor row (<=256 slots per core,
11-sigma headroom; overflow edges are summed on the host, which in
practice never triggers). Each core gathers its routed rows with two
128-row indirect DMAs and reduces them with transposed matmuls into a
single-partition [1,128] PSUM row (single-partition output keeps the
final HBM write on one DMA engine, avoiding the 16-way semaphore
trickle). The Time2Vec/rels contributions are position-sharded: core c
handles edges [256c, 256c+256) with T_DIM on partitions, so the whole
sin argument is one fused tensor_scalar, the +-pi range reduction is a
single mod op (arguments pre-biased by +9*pi on the host so trunc- and
floor-mod agree), and the masked sums are fused multiply+reduce ops.
The host adds the 8 partial [128] outputs.
"""

import sys

import numpy as np

if "/opt/trn_rl_repo" not in sys.path:
    sys.path.insert(0, "/opt/trn_rl_repo")

N_NODES = 1_000_000
E = 2048
HIDDEN = 128
P = 128
NCORES = 8
ROWS = N_NODES // NCORES  # 125000
T_DIM = 64
R_DIM = 32
EB = E // NCORES  # 256 edges per core for the t2v/rels block
NCHV = 2          # gather chunks per core
CAP = NCHV * P    # 256 routed v-row slots per core

# packed-parameter column layout ([128, F_COLS] f32)
# The linear Time2Vec channel rides as row 32 of the rels block (its
# values w0*t+b0 are a host-side affine of the shipped times), so every
# engine op can use partition ranges starting at 0 (the BIR verifier
# rejects odd partition offsets like [1:64]).
C_TM = 0                  # rows 0..63:  times block, replicated x64
C_EW = C_TM + EB          # rows 0..63:  mask block (f32), replicated x64
C_RT = C_EW + EB          # rows 0..31:  rels block, transposed [32, 256];
                          #              row 32: linear-channel values;
                          #              rows 33..63: zero
C_W = C_RT + EB           # rows 0..63:  col C_W   = w vec
                          #              col C_W+1 = b vec
                          #              col C_W+2 = w/2pi
                          #              col C_W+3 = b/2pi + magic
C_WTV = C_W + 4           # rows 0..63:  W_tv [64, 128] with row 0 zeroed
C_WRV = C_WTV + HIDDEN    # rows 0..31:  W_rv; row 32: W_tv's true row 0
C_VW = C_WRV + HIDDEN     # rows 0..127: routed-slot weights [128, NCHV]
F_COLS = C_VW + NCHV

_CACHE = {}


def _build_program():
    import concourse.bass as bass
    import concourse.tile as tile
    from concourse import bacc, mybir

    f32 = mybir.dt.float32
    i32 = mybir.dt.int32
    PI = float(np.pi)
    ALU = mybir.AluOpType

    nc = bacc.Bacc()
    v_shard = nc.declare_dram_parameter("v_shard", [ROWS, HIDDEN], f32, isOutput=False)
    idx_pc = nc.declare_dram_parameter("idx_pc", [P, NCHV], i32, isOutput=False)
    pk = nc.declare_dram_parameter("pk", [P, F_COLS], f32, isOutput=False)
    out = nc.declare_dram_parameter("out", [HIDDEN], f32, isOutput=True)

    with tile.TileContext(nc) as tc:
        with (
            tc.tile_pool(name="sb", bufs=1) as pool,
            tc.tile_pool(name="ps", bufs=1, space="PSUM") as psum,
        ):
            idx_t = pool.tile([P, NCHV], i32)
            pk_t = pool.tile([P, F_COLS], f32)
            gath = [
                pool.tile([P, HIDDEN], f32, tag=f"g{j}", name=f"gath{j}")
                for j in range(NCHV)
            ]
            te_t = pool.tile([T_DIM, EB], f32)   # arg, then sin(arg)
            u_t = pool.tile([T_DIM, EB], f32)    # arg/2pi + magic
            qm_t = pool.tile([T_DIM, EB], f32)   # -2pi * round(arg/2pi)
            r_t = pool.tile([T_DIM, EB], f32)    # range-reduced argument
            wte_t = pool.tile([T_DIM, EB], f32)  # scratch for fused mul+reduce
            wre_t = pool.tile([T_DIM, EB], f32)
            te_sum = pool.tile([T_DIM, 1], f32)
            r_sum = pool.tile([T_DIM, 1], f32)
            orow_t = pool.tile([1, HIDDEN], f32)
            ps_row = psum.tile([1, HIDDEN], f32, space="PSUM")

            # Index load first so the gathers start as early as possible.
            nc.sync.dma_start(out=idx_t[:], in_=idx_pc[:])
            for j in range(NCHV):
                nc.gpsimd.indirect_dma_start(
                    out=gath[j][:],
                    out_offset=None,
                    in_=v_shard[:],
                    in_offset=bass.IndirectOffsetOnAxis(
                        ap=idx_t[:, j : j + 1], axis=0
                    ),
                )
            nc.scalar.dma_start(out=pk_t[:], in_=pk[:])

            tm_bc = pk_t[0:T_DIM, C_TM : C_TM + EB]
            ew64 = pk_t[0:T_DIM, C_EW : C_EW + EB]
            rels_T = pk_t[0:T_DIM, C_RT : C_RT + EB]
            w_col = pk_t[0:T_DIM, C_W : C_W + 1]
            b_col = pk_t[0:T_DIM, C_W + 1 : C_W + 2]
            wtv = pk_t[0:T_DIM, C_WTV : C_WTV + HIDDEN]
            wrv = pk_t[0:T_DIM, C_WRV : C_WRV + HIDDEN]
            vw = pk_t[:, C_VW : C_VW + NCHV]

            # arg[j,e] = t[e]*w[j] + b[j]; range-reduce to [-pi, pi] for
            # the Sin table via the fp32 magic-number round:
            # r = arg - 2pi * round(arg/2pi). Row 0's sin is junk (the true
            # linear channel rides in the rels block; W_tv row 0 is zeroed).
            MAGIC = 12582912.0  # 1.5 * 2**23
            nc.vector.tensor_scalar(
                out=te_t[:], in0=tm_bc, scalar1=w_col, scalar2=b_col,
                op0=ALU.mult, op1=ALU.add,
            )
            nc.vector.tensor_scalar(
                out=u_t[:], in0=te_t[:], scalar1=1.0 / (2.0 * PI),
                scalar2=MAGIC, op0=ALU.mult, op1=ALU.add,
            )
            nc.vector.tensor_scalar(
                out=qm_t[:], in0=u_t[:], scalar1=MAGIC,
                scalar2=-2.0 * PI, op0=ALU.subtract, op1=ALU.mult,
            )
            nc.vector.tensor_tensor(
                out=r_t[:], in0=te_t[:], in1=qm_t[:], op=ALU.add
            )
            nc.scalar.activation(
                out=te_t[:], in_=r_t[:],
                func=mybir.ActivationFunctionType.Sin,
            )
            # Masked sums over the free (edge) axis.
            nc.vector.tensor_tensor(
                out=wte_t[:], in0=te_t[:], in1=ew64, op=ALU.mult
            )
            nc.vector.tensor_reduce(
                out=te_sum[:], in_=wte_t[:], axis=mybir.AxisListType.X,
                op=ALU.add,
            )
            nc.vector.tensor_tensor(
                out=wre_t[:], in0=rels_T, in1=ew64, op=ALU.mult
            )
            nc.vector.tensor_reduce(
                out=r_sum[:], in_=wre_t[:], axis=mybir.AxisListType.X,
                op=ALU.add,
            )

            # All contributions accumulate into one single-partition PSUM
            # row (out[0,h] = sum_p lhsT[p,0] * rhs[p,h]) so the final HBM
            # write is a one-descriptor DMA, ordered by operand readiness.
            nc.tensor.matmul(
                out=ps_row[:], lhsT=vw[:, 0:1], rhs=gath[0][:],
                start=True, stop=False,
            )
            nc.tensor.matmul(
                out=ps_row[:], lhsT=r_sum[:], rhs=wrv, start=False, stop=False
            )
            nc.tensor.matmul(
                out=ps_row[:], lhsT=vw[:, 1:2], rhs=gath[1][:],
                start=False, stop=False,
            )
            nc.tensor.matmul(
                out=ps_row[:], lhsT=te_sum[:], rhs=wtv, start=False, stop=True
            )
            nc.vector.tensor_copy(out=orow_t[:], in_=ps_row[:])
            nc.sync.dma_start(
                out=out[:].rearrange("(o f) -> o f", o=1), in_=orow_t[:]
            )

    nc.compile()
    return nc


def _prep_in_maps(v_, neighbors, mask, times, rels, w0, b0, w, b, Wt, We):
    PI = float(np.pi)
    v_np = np.ascontiguousarray(np.asarray(v_, dtype=np.float32))
    nb = np.asarray(neighbors).astype(np.int64).ravel()
    m = np.asarray(mask).astype(bool).ravel()
    t = np.asarray(times, dtype=np.float32).ravel()
    rels_np = np.asarray(rels, dtype=np.float32)

    Wtv = np.ascontiguousarray(
        np.asarray(Wt, dtype=np.float32)[:, 2 * HIDDEN : 3 * HIDDEN]
    )
    Wrv = np.ascontiguousarray(
        np.asarray(We, dtype=np.float32)[:, 2 * HIDDEN : 3 * HIDDEN]
    )
    w0f = np.float32(np.asarray(w0))
    b0f = np.float32(np.asarray(b0))
    wf = np.empty(T_DIM, np.float32)
    wf[0] = w0f  # row 0 is a junk sin channel (W_tv row 0 zeroed)
    wf[1:] = np.asarray(w, dtype=np.float32).ravel()
    bf = np.empty(T_DIM, np.float32)
    bf[0] = b0f
    bf[1:] = np.asarray(b, dtype=np.float32).ravel()

    owner = nb // ROWS
    local = (nb - owner * ROWS).astype(np.int32)

    in_maps = []
    spill = np.zeros(HIDDEN, np.float64)
    for c in range(NCORES):
        sel = np.nonzero(m & (owner == c))[0]
        if len(sel) > CAP:  # ~11 sigma; handled on host for correctness
            over = sel[CAP:]
            sel = sel[:CAP]
            spill += v_np[nb[over]].sum(axis=0, dtype=np.float64)
        n = len(sel)
        idx = np.zeros(CAP, np.int32)
        wts = np.zeros(CAP, np.float32)
        idx[:n] = local[sel]
        wts[:n] = 1.0
        # slot s <-> (chunk j = s // P, partition p = s % P)
        idx_pc = np.ascontiguousarray(idx.reshape(NCHV, P).T)
        wts_pc = np.ascontiguousarray(wts.reshape(NCHV, P).T)

        blk = slice(EB * c, EB * (c + 1))
        pk = np.zeros((P, F_COLS), np.float32)
        pk[0:T_DIM, C_TM : C_TM + EB] = t[blk][None, :]
        pk[0:T_DIM, C_EW : C_EW + EB] = m[blk].astype(np.float32)[None, :]
        pk[0:R_DIM, C_RT : C_RT + EB] = rels_np[blk].T
        pk[R_DIM, C_RT : C_RT + EB] = w0f * t[blk] + b0f  # linear channel
        pk[0:T_DIM, C_W] = wf
        pk[0:T_DIM, C_W + 1] = bf
        pk[0:T_DIM, C_W + 2] = wf / np.float32(2.0 * PI)
        pk[0:T_DIM, C_W + 3] = bf / np.float32(2.0 * PI) + np.float32(12582912.0)
        pk[0:T_DIM, C_WTV : C_WTV + HIDDEN] = Wtv
        pk[0, C_WTV : C_WTV + HIDDEN] = 0.0  # linear channel moved to rels row
        pk[0:R_DIM, C_WRV : C_WRV + HIDDEN] = Wrv
        pk[R_DIM, C_WRV : C_WRV + HIDDEN] = Wtv[0]
        pk[:, C_VW : C_VW + NCHV] = wts_pc

        in_maps.append(
            {
                "v_shard": v_np[c * ROWS : (c + 1) * ROWS],
                "idx_pc": idx_pc,
                "pk": pk,
            }
        )
    return in_maps, spill


def kernel(
    k_,
    q_,
    v_,
    neighbors,
    nid,
    mask,
    start_t,
    times,
    rels,
    t2v_w0,
    t2v_b0,
    t2v_w,
    t2v_b,
    time_kqv_w,
    edge_kqv_w,
):
    from concourse.bass_utils import run_bass_kernel_spmd

    nc = _CACHE.get("nc")
    if nc is None:
        nc = _build_program()
        _CACHE["nc"] = nc

    in_maps, spill = _prep_in_maps(
        v_, neighbors, mask, times, rels, t2v_w0, t2v_b0, t2v_w, t2v_b,
        time_kqv_w, edge_kqv_w,
    )
    res = run_bass_kernel_spmd(nc, in_maps, list(range(NCORES)))
    total = np.sum(
        np.stack([r["out"] for r in res.results]).astype(np.float64), axis=0
    )
    return (total + spill).astype(np.float32).reshape(1, HIDDEN)


# revision 14
# speedup vs baseline: 2.3744x; 1.0317x over previous
"""Bass/Trainium2 SPMD kernel for nn_JittableSelfAttention_Rels.

The reference's softmax is over the singleton query dim, so attention
weights are identically 1 and the output reduces to

    out[1,128] = sum_{e: mask[e]} ( v_[neighbors[e]]
                                    + t2v(times[e]) @ W_tv
                                    + rels[e] @ W_rv )

where W_tv / W_rv are the v-thirds of time_kqv_w / edge_kqv_w.

Sharding: v_ is split row-wise across 8 cores. The host routes each
masked edge to the core owning its neighbor row (<=256 slots per core,
11-sigma headroom; overflow edges are summed on the host, which in
practice never triggers). Each core gathers its routed rows with two
128-row indirect DMAs and reduces them with transposed matmuls into a
single-partition [1,128] PSUM row (single-partition output keeps the
final HBM write on one DMA engine, avoiding the 16-way semaphore
trickle). The Time2Vec/rels contributions are position-sharded: core c
handles edges [256c, 256c+256) with T_DIM on partitions, so the whole
sin argument is one fused tensor_scalar, the +-pi range reduction is a
single mod op (arguments pre-biased by +9*pi on the host so trunc- and
floor-mod agree), and the masked sums are fused multiply+reduce ops.
The host adds the 8 partial [128] outputs.
"""

import sys

import numpy as np

if "/opt/trn_rl_repo" not in sys.path:
    sys.path.insert(0, "/opt/trn_rl_repo")

N_NODES = 1_000_000
E = 2048
HIDDEN = 128
P = 128
NCORES = 8
ROWS = N_NODES // NCORES  # 125000
T_DIM = 64
R_DIM = 32
EB = E // NCORES  # 256 edges per core for the t2v/rels block
NCHV = 2          # gather chunks per core
CAP = NCHV * P    # 256 routed v-row slots per core

# packed-parameter column layout ([128, F_COLS] f32)
# The linear Time2Vec channel rides as row 32 of the rels block (its
# values w0*t+b0 are a host-side affine of the shipped times), so every
# engine op can use partition ranges starting at 0 (the BIR verifier
# rejects odd partition offsets like [1:64]).
C_TM = 0                  # rows 0..63:  times block, replicated x64
C_EW = C_TM + EB          # rows 0..63:  mask block (f32), replicated x64
C_RT = C_EW + EB          # rows 0..31:  rels block, transposed [32, 256];
                          #              row 32: linear-channel values;
                          #              rows 33..63: zero
C_W = C_RT + EB           # rows 0..63:  col C_W   = w vec
                          #              col C_W+1 = b vec
                          #              col C_W+2 = w/2pi
                          #              col C_W+3 = b/2pi + magic
C_WTV = C_W + 4           # rows 0..63:  W_tv [64, 128] with row 0 zeroed
C_WRV = C_WTV + HIDDEN    # rows 0..31:  W_rv; row 32: W_tv's true row 0
C_VW = C_WRV + HIDDEN     # rows 0..127: routed-slot weights [128, NCHV]
F_COLS = C_VW + NCHV

_CACHE = {}


def _build_program():
    import concourse.bass as bass
    import concourse.tile as tile
    from concourse import bacc, mybir

    f32 = mybir.dt.float32
    i32 = mybir.dt.int32
    PI = float(np.pi)
    ALU = mybir.AluOpType

    nc = bacc.Bacc()
    v_shard = nc.declare_dram_parameter("v_shard", [ROWS, HIDDEN], f32, isOutput=False)
    idx_pc = nc.declare_dram_parameter("idx_pc", [P, NCHV], i32, isOutput=False)
    pk = nc.declare_dram_parameter("pk", [P, F_COLS], f32, isOutput=False)
    out = nc.declare_dram_parameter("out", [HIDDEN], f32, isOutput=True)

    with tile.TileContext(nc) as tc:
        with (
            tc.tile_pool(name="sb", bufs=1) as pool,
            tc.tile_pool(name="ps", bufs=1, space="PSUM") as psum,
        ):
            idx_t = pool.tile([P, NCHV], i32)
            pk_t = pool.tile([P, F_COLS], f32)
            gath = [
                pool.tile([P, HIDDEN], f32, tag=f"g{j}", name=f"gath{j}")
                for j in range(NCHV)
            ]
            te_t = pool.tile([T_DIM, EB], f32)   # arg, then sin(arg)
            u_t = pool.tile([T_DIM, EB], f32)    # arg/2pi + magic
            qm_t = pool.tile([T_DIM, EB], f32)   # -2pi * round(arg/2pi)
            r_t = pool.tile([T_DIM, EB], f32)    # range-reduced argument
            wte_t = pool.tile([T_DIM, EB], f32)  # scratch for fused mul+reduce
            wre_t = pool.tile([T_DIM, EB], f32)
            te_sum = pool.tile([T_DIM, 1], f32)
            r_sum = pool.tile([T_DIM, 1], f32)
            orow_t = pool.tile([1, HIDDEN], f32)
            ps_row = psum.tile([1, HIDDEN], f32, space="PSUM")

            # Index load first so the gathers start as early as possible.
            nc.sync.dma_start(out=idx_t[:], in_=idx_pc[:])
            for j in range(NCHV):
                nc.gpsimd.indirect_dma_start(
                    out=gath[j][:],
                    out_offset=None,
                    in_=v_shard[:],
                    in_offset=bass.IndirectOffsetOnAxis(
                        ap=idx_t[:, j : j + 1], axis=0
                    ),
                )
            nc.sync.dma_start(out=pk_t[:], in_=pk[:])

            tm_bc = pk_t[0:T_DIM, C_TM : C_TM + EB]
            ew64 = pk_t[0:T_DIM, C_EW : C_EW + EB]
            rels_T = pk_t[0:T_DIM, C_RT : C_RT + EB]
            w_col = pk_t[0:T_DIM, C_W : C_W + 1]
            b_col = pk_t[0:T_DIM, C_W + 1 : C_W + 2]
            wtv = pk_t[0:T_DIM, C_WTV : C_WTV + HIDDEN]
            wrv = pk_t[0:T_DIM, C_WRV : C_WRV + HIDDEN]
            vw = pk_t[:, C_VW : C_VW + NCHV]

            # arg[j,e] = t[e]*w[j] + b[j]; range-reduce to [-pi, pi] for
            # the Sin table via the fp32 magic-number round:
            # r = arg - 2pi * round(arg/2pi). Row 0's sin is junk (the true
            # linear channel rides in the rels block; W_tv row 0 is zeroed).
            MAGIC = 12582912.0  # 1.5 * 2**23
            nc.vector.tensor_scalar(
                out=te_t[:], in0=tm_bc, scalar1=w_col, scalar2=b_col,
                op0=ALU.mult, op1=ALU.add,
            )
            nc.vector.tensor_scalar(
                out=u_t[:], in0=te_t[:], scalar1=1.0 / (2.0 * PI),
                scalar2=MAGIC, op0=ALU.mult, op1=ALU.add,
            )
            nc.vector.tensor_scalar(
                out=qm_t[:], in0=u_t[:], scalar1=MAGIC,
                scalar2=-2.0 * PI, op0=ALU.subtract, op1=ALU.mult,
            )
            nc.vector.tensor_tensor(
                out=r_t[:], in0=te_t[:], in1=qm_t[:], op=ALU.add
            )
            nc.scalar.activation(
                out=te_t[:], in_=r_t[:],
                func=mybir.ActivationFunctionType.Sin,
            )
            # Masked sums over the free (edge) axis.
            nc.vector.tensor_tensor(
                out=wte_t[:], in0=te_t[:], in1=ew64, op=ALU.mult
            )
            nc.vector.tensor_reduce(
                out=te_sum[:], in_=wte_t[:], axis=mybir.AxisListType.X,
                op=ALU.add,
            )
            nc.vector.tensor_tensor(
                out=wre_t[:], in0=rels_T, in1=ew64, op=ALU.mult
            )
            nc.vector.tensor_reduce(
                out=r_sum[:], in_=wre_t[:], axis=mybir.AxisListType.X,
                op=ALU.add,
            )

            # All contributions accumulate into one single-partition PSUM
            # row (out[0,h] = sum_p lhsT[p,0] * rhs[p,h]) so the final HBM
            # write is a one-descriptor DMA, ordered by operand readiness.
            nc.tensor.matmul(
                out=ps_row[:], lhsT=vw[:, 0:1], rhs=gath[0][:],
                start=True, stop=False,
            )
            nc.tensor.matmul(
                out=ps_row[:], lhsT=r_sum[:], rhs=wrv, start=False, stop=False
            )
            nc.tensor.matmul(
                out=ps_row[:], lhsT=vw[:, 1:2], rhs=gath[1][:],
                start=False, stop=False,
            )
            nc.tensor.matmul(
                out=ps_row[:], lhsT=te_sum[:], rhs=wtv, start=False, stop=True
            )
            nc.vector.tensor_copy(out=orow_t[:], in_=ps_row[:])
            nc.sync.dma_start(
                out=out[:].rearrange("(o f) -> o f", o=1), in_=orow_t[:]
            )

    nc.compile()
    return nc


def _prep_in_maps(v_, neighbors, mask, times, rels, w0, b0, w, b, Wt, We):
    PI = float(np.pi)
    v_np = np.ascontiguousarray(np.asarray(v_, dtype=np.float32))
    nb = np.asarray(neighbors).astype(np.int64).ravel()
    m = np.asarray(mask).astype(bool).ravel()
    t = np.asarray(times, dtype=np.float32).ravel()
    rels_np = np.asarray(rels, dtype=np.float32)

    Wtv = np.ascontiguousarray(
        np.asarray(Wt, dtype=np.float32)[:, 2 * HIDDEN : 3 * HIDDEN]
    )
    Wrv = np.ascontiguousarray(
        np.asarray(We, dtype=np.float32)[:, 2 * HIDDEN : 3 * HIDDEN]
    )
    w0f = np.float32(np.asarray(w0))
    b0f = np.float32(np.asarray(b0))
    wf = np.empty(T_DIM, np.float32)
    wf[0] = w0f  # row 0 is a junk sin channel (W_tv row 0 zeroed)
    wf[1:] = np.asarray(w, dtype=np.float32).ravel()
    bf = np.empty(T_DIM, np.float32)
    bf[0] = b0f
    bf[1:] = np.asarray(b, dtype=np.float32).ravel()

    owner = nb // ROWS
    local = (nb - owner * ROWS).astype(np.int32)

    in_maps = []
    spill = np.zeros(HIDDEN, np.float64)
    for c in range(NCORES):
        sel = np.nonzero(m & (owner == c))[0]
        if len(sel) > CAP:  # ~11 sigma; handled on host for correctness
            over = sel[CAP:]
            sel = sel[:CAP]
            spill += v_np[nb[over]].sum(axis=0, dtype=np.float64)
        n = len(sel)
        idx = np.zeros(CAP, np.int32)
        wts = np.zeros(CAP, np.float32)
        idx[:n] = local[sel]
        wts[:n] = 1.0
        # slot s <-> (chunk j = s // P, partition p = s % P)
        idx_pc = np.ascontiguousarray(idx.reshape(NCHV, P).T)
        wts_pc = np.ascontiguousarray(wts.reshape(NCHV, P).T)

        blk = slice(EB * c, EB * (c + 1))
        pk = np.zeros((P, F_COLS), np.float32)
        pk[0:T_DIM, C_TM : C_TM + EB] = t[blk][None, :]
        pk[0:T_DIM, C_EW : C_EW + EB] = m[blk].astype(np.float32)[None, :]
        pk[0:R_DIM, C_RT : C_RT + EB] = rels_np[blk].T
        pk[R_DIM, C_RT : C_RT + EB] = w0f * t[blk] + b0f  # linear channel
        pk[0:T_DIM, C_W] = wf
        pk[0:T_DIM, C_W + 1] = bf
        pk[0:T_DIM, C_W + 2] = wf / np.float32(2.0 * PI)
        pk[0:T_DIM, C_W + 3] = bf / np.float32(2.0 * PI) + np.float32(12582912.0)
        pk[0:T_DIM, C_WTV : C_WTV + HIDDEN] = Wtv
        pk[0, C_WTV : C_WTV + HIDDEN] = 0.0  # linear channel moved to rels row
        pk[0:R_DIM, C_WRV : C_WRV + HIDDEN] = Wrv
        pk[R_DIM, C_WRV : C_WRV + HIDDEN] = Wtv[0]
        pk[:, C_VW : C_VW + NCHV] = wts_pc

        in_maps.append(
            {
                "v_shard": v_np[c * ROWS : (c + 1) * ROWS],
                "idx_pc": idx_pc,
                "pk": pk,
            }
        )
    return in_maps, spill


def kernel(
    k_,
    q_,
    v_,
    neighbors,
    nid,
    mask,
    start_t,
    times,
    rels,
    t2v_w0,
    t2v_b0,
    t2v_w,
    t2v_b,
    time_kqv_w,
    edge_kqv_w,
):
    from concourse.bass_utils import run_bass_kernel_spmd

    nc = _CACHE.get("nc")
    if nc is None:
        nc = _build_program()
        _CACHE["nc"] = nc

    in_maps, spill = _prep_in_maps(
        v_, neighbors, mask, times, rels, t2v_w0, t2v_b0, t2v_w, t2v_b,
        time_kqv_w, edge_kqv_w,
    )
    res = run_bass_kernel_spmd(nc, in_maps, list(range(NCORES)))
    total = np.sum(
        np.stack([r["out"] for r in res.results]).astype(np.float64), axis=0
    )
    return (total + spill).astype(np.float32).reshape(1, HIDDEN)


# revision 15
# speedup vs baseline: 2.7450x; 1.1561x over previous
"""Bass/Trainium2 SPMD kernel for nn_JittableSelfAttention_Rels.

The reference's softmax is over the singleton query dim, so attention
weights are identically 1 and the output reduces to

    out[1,128] = sum_{e: mask[e]} ( v_[neighbors[e]]
                                    + t2v(times[e]) @ W_tv
                                    + rels[e] @ W_rv )

where W_tv / W_rv are the v-thirds of time_kqv_w / edge_kqv_w.

Sharding: v_ is split row-wise across 8 cores. The host routes each
masked edge to the core owning its neighbor row (<=256 slots per core,
11-sigma headroom; overflow edges are summed on the host, which in
practice never triggers). Each core gathers its routed rows with two
128-row indirect DMAs and reduces them with transposed matmuls into a
single-partition [1,128] PSUM row (single-partition output keeps the
final HBM write on one DMA engine, avoiding the 16-way semaphore
trickle). The Time2Vec/rels contributions are position-sharded: core c
handles edges [256c, 256c+256) with T_DIM on partitions, so the whole
sin argument is one fused tensor_scalar, the +-pi range reduction is a
single mod op (arguments pre-biased by +9*pi on the host so trunc- and
floor-mod agree), and the masked sums are fused multiply+reduce ops.
The host adds the 8 partial [128] outputs.
"""

import sys

import numpy as np

if "/opt/trn_rl_repo" not in sys.path:
    sys.path.insert(0, "/opt/trn_rl_repo")

N_NODES = 1_000_000
E = 2048
HIDDEN = 128
P = 128
NCORES = 8
ROWS = N_NODES // NCORES  # 125000
T_DIM = 64
R_DIM = 32
EB = E // NCORES  # 256 edges per core for the t2v/rels block
NCHV = 2          # gather chunks per core
CAP = NCHV * P    # 256 routed v-row slots per core

# Parameters ride in three packed tensors, split by when each block is
# first needed on device (three DMAs -> earlier completion receipts for
# the early blocks). The linear Time2Vec channel rides as row 32 of the
# rels block (its values w0*t+b0 are a host-side affine of the shipped
# times), so every engine op can use partition ranges starting at 0
# (the BIR verifier rejects odd partition offsets like [1:64]).
# pk1 [64, EB+2]:  cols 0..255 times (replicated x64), col 256 w, col 257 b
# pk2 [64, 2*EB]:  cols 0..255 mask f32 (replicated x64);
#                  cols 256..511 rels transposed [32 rows] + linear row 32
# pk3 [128, 2*HIDDEN+NCHV]: cols 0..127 W_tv (row 0 zeroed, rows 0..63),
#                  cols 128..255 W_rv rows 0..31 + W_tv row 0 at row 32,
#                  cols 256..257 routed-slot weights (all 128 rows)
F1 = EB + 2
F2 = 2 * EB
F3 = 2 * HIDDEN + NCHV

_CACHE = {}


def _build_program():
    import concourse.bass as bass
    import concourse.tile as tile
    from concourse import bacc, mybir

    f32 = mybir.dt.float32
    i32 = mybir.dt.int32
    PI = float(np.pi)
    ALU = mybir.AluOpType

    nc = bacc.Bacc()
    v_shard = nc.declare_dram_parameter("v_shard", [ROWS, HIDDEN], f32, isOutput=False)
    idx_pc = nc.declare_dram_parameter("idx_pc", [P, NCHV], i32, isOutput=False)
    pk1 = nc.declare_dram_parameter("pk1", [T_DIM, F1], f32, isOutput=False)
    pk2 = nc.declare_dram_parameter("pk2", [T_DIM, F2], f32, isOutput=False)
    pk3 = nc.declare_dram_parameter("pk3", [P, F3], f32, isOutput=False)
    out = nc.declare_dram_parameter("out", [HIDDEN], f32, isOutput=True)

    with tile.TileContext(nc) as tc:
        with (
            tc.tile_pool(name="sb", bufs=1) as pool,
            tc.tile_pool(name="ps", bufs=1, space="PSUM") as psum,
        ):
            idx_t = pool.tile([P, NCHV], i32)
            pk1_t = pool.tile([T_DIM, F1], f32)
            pk2_t = pool.tile([T_DIM, F2], f32)
            pk3_t = pool.tile([P, F3], f32)
            gath = [
                pool.tile([P, HIDDEN], f32, tag=f"g{j}", name=f"gath{j}")
                for j in range(NCHV)
            ]
            te_t = pool.tile([T_DIM, EB], f32)   # arg, then sin(arg)
            u_t = pool.tile([T_DIM, EB], f32)    # arg/2pi + magic
            qm_t = pool.tile([T_DIM, EB], f32)   # -2pi * round(arg/2pi)
            r_t = pool.tile([T_DIM, EB], f32)    # range-reduced argument
            wte_t = pool.tile([T_DIM, EB], f32)  # scratch for fused mul+reduce
            wre_t = pool.tile([T_DIM, EB], f32)
            te_sum = pool.tile([T_DIM, 1], f32)
            r_sum = pool.tile([T_DIM, 1], f32)
            orow_t = pool.tile([1, HIDDEN], f32)
            ps_row = psum.tile([1, HIDDEN], f32, space="PSUM")

            # Index load first so the gathers start as early as possible.
            nc.sync.dma_start(out=idx_t[:], in_=idx_pc[:])
            for j in range(NCHV):
                nc.gpsimd.indirect_dma_start(
                    out=gath[j][:],
                    out_offset=None,
                    in_=v_shard[:],
                    in_offset=bass.IndirectOffsetOnAxis(
                        ap=idx_t[:, j : j + 1], axis=0
                    ),
                )
            nc.sync.dma_start(out=pk1_t[:], in_=pk1[:])
            nc.sync.dma_start(out=pk2_t[:], in_=pk2[:])
            nc.sync.dma_start(out=pk3_t[:], in_=pk3[:])

            tm_bc = pk1_t[:, 0:EB]
            w_col = pk1_t[:, EB : EB + 1]
            b_col = pk1_t[:, EB + 1 : EB + 2]
            ew64 = pk2_t[:, 0:EB]
            rels_T = pk2_t[:, EB : 2 * EB]
            wtv = pk3_t[0:T_DIM, 0:HIDDEN]
            wrv = pk3_t[0:T_DIM, HIDDEN : 2 * HIDDEN]
            vw = pk3_t[:, 2 * HIDDEN : 2 * HIDDEN + NCHV]

            # arg[j,e] = t[e]*w[j] + b[j]; range-reduce to [-pi, pi] for
            # the Sin table via the fp32 magic-number round:
            # r = arg - 2pi * round(arg/2pi). Row 0's sin is junk (the true
            # linear channel rides in the rels block; W_tv row 0 is zeroed).
            MAGIC = 12582912.0  # 1.5 * 2**23
            nc.vector.tensor_scalar(
                out=te_t[:], in0=tm_bc, scalar1=w_col, scalar2=b_col,
                op0=ALU.mult, op1=ALU.add,
            )
            nc.vector.tensor_scalar(
                out=u_t[:], in0=te_t[:], scalar1=1.0 / (2.0 * PI),
                scalar2=MAGIC, op0=ALU.mult, op1=ALU.add,
            )
            nc.vector.tensor_scalar(
                out=qm_t[:], in0=u_t[:], scalar1=MAGIC,
                scalar2=-2.0 * PI, op0=ALU.subtract, op1=ALU.mult,
            )
            nc.vector.tensor_tensor(
                out=r_t[:], in0=te_t[:], in1=qm_t[:], op=ALU.add
            )
            nc.scalar.activation(
                out=te_t[:], in_=r_t[:],
                func=mybir.ActivationFunctionType.Sin,
            )
            # Masked sums over the free (edge) axis.
            nc.vector.tensor_tensor(
                out=wte_t[:], in0=te_t[:], in1=ew64, op=ALU.mult
            )
            nc.vector.tensor_reduce(
                out=te_sum[:], in_=wte_t[:], axis=mybir.AxisListType.X,
                op=ALU.add,
            )
            nc.vector.tensor_tensor(
                out=wre_t[:], in0=rels_T, in1=ew64, op=ALU.mult
            )
            nc.vector.tensor_reduce(
                out=r_sum[:], in_=wre_t[:], axis=mybir.AxisListType.X,
                op=ALU.add,
            )

            # All contributions accumulate into one single-partition PSUM
            # row (out[0,h] = sum_p lhsT[p,0] * rhs[p,h]) so the final HBM
            # write is a one-descriptor DMA, ordered by operand readiness.
            nc.tensor.matmul(
                out=ps_row[:], lhsT=vw[:, 0:1], rhs=gath[0][:],
                start=True, stop=False,
            )
            nc.tensor.matmul(
                out=ps_row[:], lhsT=r_sum[:], rhs=wrv, start=False, stop=False
            )
            nc.tensor.matmul(
                out=ps_row[:], lhsT=te_sum[:], rhs=wtv, start=False, stop=False
            )
            nc.tensor.matmul(
                out=ps_row[:], lhsT=vw[:, 1:2], rhs=gath[1][:],
                start=False, stop=True,
            )
            nc.vector.tensor_copy(out=orow_t[:], in_=ps_row[:])
            nc.sync.dma_start(
                out=out[:].rearrange("(o f) -> o f", o=1), in_=orow_t[:]
            )

    nc.compile()
    return nc


def _prep_in_maps(v_, neighbors, mask, times, rels, w0, b0, w, b, Wt, We):
    PI = float(np.pi)
    v_np = np.ascontiguousarray(np.asarray(v_, dtype=np.float32))
    nb = np.asarray(neighbors).astype(np.int64).ravel()
    m = np.asarray(mask).astype(bool).ravel()
    t = np.asarray(times, dtype=np.float32).ravel()
    rels_np = np.asarray(rels, dtype=np.float32)

    Wtv = np.ascontiguousarray(
        np.asarray(Wt, dtype=np.float32)[:, 2 * HIDDEN : 3 * HIDDEN]
    )
    Wrv = np.ascontiguousarray(
        np.asarray(We, dtype=np.float32)[:, 2 * HIDDEN : 3 * HIDDEN]
    )
    w0f = np.float32(np.asarray(w0))
    b0f = np.float32(np.asarray(b0))
    wf = np.empty(T_DIM, np.float32)
    wf[0] = w0f  # row 0 is a junk sin channel (W_tv row 0 zeroed)
    wf[1:] = np.asarray(w, dtype=np.float32).ravel()
    bf = np.empty(T_DIM, np.float32)
    bf[0] = b0f
    bf[1:] = np.asarray(b, dtype=np.float32).ravel()

    owner = nb // ROWS
    local = (nb - owner * ROWS).astype(np.int32)

    in_maps = []
    spill = np.zeros(HIDDEN, np.float64)
    for c in range(NCORES):
        sel = np.nonzero(m & (owner == c))[0]
        if len(sel) > CAP:  # ~11 sigma; handled on host for correctness
            over = sel[CAP:]
            sel = sel[:CAP]
            spill += v_np[nb[over]].sum(axis=0, dtype=np.float64)
        n = len(sel)
        idx = np.zeros(CAP, np.int32)
        wts = np.zeros(CAP, np.float32)
        idx[:n] = local[sel]
        wts[:n] = 1.0
        # slot s <-> (chunk j = s // P, partition p = s % P)
        idx_pc = np.ascontiguousarray(idx.reshape(NCHV, P).T)
        wts_pc = np.ascontiguousarray(wts.reshape(NCHV, P).T)

        blk = slice(EB * c, EB * (c + 1))
        pk1 = np.zeros((T_DIM, F1), np.float32)
        pk1[:, 0:EB] = t[blk][None, :]
        pk1[:, EB] = wf
        pk1[:, EB + 1] = bf
        pk2 = np.zeros((T_DIM, F2), np.float32)
        pk2[:, 0:EB] = m[blk].astype(np.float32)[None, :]
        pk2[0:R_DIM, EB : 2 * EB] = rels_np[blk].T
        pk2[R_DIM, EB : 2 * EB] = w0f * t[blk] + b0f  # linear channel
        pk3 = np.zeros((P, F3), np.float32)
        pk3[0:T_DIM, 0:HIDDEN] = Wtv
        pk3[0, 0:HIDDEN] = 0.0  # linear channel moved to the rels row
        pk3[0:R_DIM, HIDDEN : 2 * HIDDEN] = Wrv
        pk3[R_DIM, HIDDEN : 2 * HIDDEN] = Wtv[0]
        pk3[:, 2 * HIDDEN : 2 * HIDDEN + NCHV] = wts_pc

        in_maps.append(
            {
                "v_shard": v_np[c * ROWS : (c + 1) * ROWS],
                "idx_pc": idx_pc,
                "pk1": pk1,
                "pk2": pk2,
                "pk3": pk3,
            }
        )
    return in_maps, spill


def kernel(
    k_,
    q_,
    v_,
    neighbors,
    nid,
    mask,
    start_t,
    times,
    rels,
    t2v_w0,
    t2v_b0,
    t2v_w,
    t2v_b,
    time_kqv_w,
    edge_kqv_w,
):
    from concourse.bass_utils import run_bass_kernel_spmd

    nc = _CACHE.get("nc")
    if nc is None:
        nc = _build_program()
        _CACHE["nc"] = nc

    in_maps, spill = _prep_in_maps(
        v_, neighbors, mask, times, rels, t2v_w0, t2v_b0, t2v_w, t2v_b,
        time_kqv_w, edge_kqv_w,
    )
    res = run_bass_kernel_spmd(nc, in_maps, list(range(NCORES)))
    total = np.sum(
        np.stack([r["out"] for r in res.results]).astype(np.float64), axis=0
    )
    return (total + spill).astype(np.float32).reshape(1, HIDDEN)


# revision 17
# speedup vs baseline: 2.9353x; 1.0693x over previous
"""Bass/Trainium2 SPMD kernel for nn_JittableSelfAttention_Rels.

The reference's softmax is over the singleton query dim, so attention
weights are identically 1 and the output reduces to

    out[1,128] = sum_{e: mask[e]} ( v_[neighbors[e]]
                                    + t2v(times[e]) @ W_tv
                                    + rels[e] @ W_rv )

where W_tv / W_rv are the v-thirds of time_kqv_w / edge_kqv_w.

Sharding: v_ is split row-wise across 8 cores. The host routes each
masked edge to the core owning its neighbor row (<=256 slots per core,
11-sigma headroom; overflow edges are summed on the host, which in
practice never triggers). Each core gathers its routed rows with two
128-row indirect DMAs and reduces them with transposed matmuls into a
single-partition [1,128] PSUM row (single-partition output keeps the
final HBM write on one DMA engine, avoiding the 16-way semaphore
trickle). The Time2Vec/rels contributions are position-sharded: core c
handles edges [256c, 256c+256) with T_DIM on partitions, so the whole
sin argument is one fused tensor_scalar, the +-pi range reduction is a
single mod op (arguments pre-biased by +9*pi on the host so trunc- and
floor-mod agree), and the masked sums are fused multiply+reduce ops.
The host adds the 8 partial [128] outputs.
"""

import sys

import numpy as np

if "/opt/trn_rl_repo" not in sys.path:
    sys.path.insert(0, "/opt/trn_rl_repo")

N_NODES = 1_000_000
E = 2048
HIDDEN = 128
P = 128
NCORES = 8
ROWS = N_NODES // NCORES  # 125000
T_DIM = 64
R_DIM = 32
EB = E // NCORES  # 256 edges per core for the t2v/rels block
NCHV = 1          # gather chunks per core
CAP = NCHV * P    # routed v-row slots per core; statistical overflow
                  # (expected ~35 rows across all cores) is summed on the
                  # host via the spill path, which also guarantees
                  # correctness for any input distribution.

# Masked-out edges are neutralized on the HOST: their times are set to 0
# (so each sin channel contributes exactly sin(b_j), corrected once
# globally on the host) and their rels/linear rows are zeroed. This
# removes both mask-multiply ops from the device.
#
# Parameters ride in two packed tensors plus the index table, split by
# when each block is first needed on device. The linear Time2Vec channel
# rides as row 32 of the rels block (its values w0*t+b0 are a host-side
# affine of the shipped times), so every engine op can use partition
# ranges starting at 0 (the BIR verifier rejects odd partition offsets).
# pkA [64, EB+2+EB]: cols 0..255 times (replicated x64, masked-out -> 0),
#                    col 256 w, col 257 b,
#                    cols 258..513 rels transposed [32 rows] + linear
#                    row 32 (masked-out edges zeroed)
# pk3 [128, 2*HIDDEN+NCHV]: cols 0..127 W_tv (row 0 zeroed, rows 0..63),
#                    cols 128..255 W_rv rows 0..31 + W_tv row 0 at row 32,
#                    col 256 routed-slot weights (all 128 rows)
FA = 2 * EB + 2
F3 = 2 * HIDDEN + NCHV

_CACHE = {}


def _build_program():
    import concourse.bass as bass
    import concourse.tile as tile
    from concourse import bacc, mybir

    f32 = mybir.dt.float32
    i32 = mybir.dt.int32
    PI = float(np.pi)
    ALU = mybir.AluOpType

    nc = bacc.Bacc()
    v_shard = nc.declare_dram_parameter("v_shard", [ROWS, HIDDEN], f32, isOutput=False)
    idx_pc = nc.declare_dram_parameter("idx_pc", [P, NCHV], i32, isOutput=False)
    pkA = nc.declare_dram_parameter("pkA", [T_DIM, FA], f32, isOutput=False)
    pk3 = nc.declare_dram_parameter("pk3", [P, F3], f32, isOutput=False)
    out = nc.declare_dram_parameter("out", [HIDDEN], f32, isOutput=True)

    with tile.TileContext(nc) as tc:
        with (
            tc.tile_pool(name="sb", bufs=1) as pool,
            tc.tile_pool(name="ps", bufs=1, space="PSUM") as psum,
        ):
            idx_t = pool.tile([P, NCHV], i32)
            pkA_t = pool.tile([T_DIM, FA], f32)
            pk3_t = pool.tile([P, F3], f32)
            gath = [
                pool.tile([P, HIDDEN], f32, tag=f"g{j}", name=f"gath{j}")
                for j in range(NCHV)
            ]
            te_t = pool.tile([T_DIM, EB], f32)   # arg, then sin(arg)
            u_t = pool.tile([T_DIM, EB], f32)    # arg/2pi + magic
            qm_t = pool.tile([T_DIM, EB], f32)   # -2pi * round(arg/2pi)
            r_t = pool.tile([T_DIM, EB], f32)    # range-reduced argument
            te_sum = pool.tile([T_DIM, 1], f32)
            r_sum = pool.tile([T_DIM, 1], f32)
            orow_t = pool.tile([1, HIDDEN], f32)
            ps_row = psum.tile([1, HIDDEN], f32, space="PSUM")

            # Index load first so the gathers start as early as possible.
            nc.sync.dma_start(out=idx_t[:], in_=idx_pc[:])
            for j in range(NCHV):
                nc.gpsimd.indirect_dma_start(
                    out=gath[j][:],
                    out_offset=None,
                    in_=v_shard[:],
                    in_offset=bass.IndirectOffsetOnAxis(
                        ap=idx_t[:, j : j + 1], axis=0
                    ),
                )
            nc.sync.dma_start(out=pkA_t[:], in_=pkA[:])
            nc.sync.dma_start(out=pk3_t[:], in_=pk3[:])

            tm_bc = pkA_t[:, 0:EB]
            w_col = pkA_t[:, EB : EB + 1]
            b_col = pkA_t[:, EB + 1 : EB + 2]
            rels_T = pkA_t[:, EB + 2 : EB + 2 + EB]
            wtv = pk3_t[0:T_DIM, 0:HIDDEN]
            wrv = pk3_t[0:T_DIM, HIDDEN : 2 * HIDDEN]
            vw = pk3_t[:, 2 * HIDDEN : 2 * HIDDEN + NCHV]

            # arg[j,e] = t[e]*w[j] + b[j]; range-reduce to [-pi, pi] for
            # the Sin table via the fp32 magic-number round:
            # r = arg - 2pi * round(arg/2pi). Row 0's sin is junk (the true
            # linear channel rides in the rels block; W_tv row 0 is zeroed).
            MAGIC = 12582912.0  # 1.5 * 2**23
            nc.vector.tensor_scalar(
                out=te_t[:], in0=tm_bc, scalar1=w_col, scalar2=b_col,
                op0=ALU.mult, op1=ALU.add,
            )
            nc.vector.tensor_scalar(
                out=u_t[:], in0=te_t[:], scalar1=1.0 / (2.0 * PI),
                scalar2=MAGIC, op0=ALU.mult, op1=ALU.add,
            )
            nc.vector.tensor_scalar(
                out=qm_t[:], in0=u_t[:], scalar1=MAGIC,
                scalar2=-2.0 * PI, op0=ALU.subtract, op1=ALU.mult,
            )
            nc.vector.tensor_tensor(
                out=r_t[:], in0=te_t[:], in1=qm_t[:], op=ALU.add
            )
            nc.scalar.activation(
                out=te_t[:], in_=r_t[:],
                func=mybir.ActivationFunctionType.Sin,
            )
            # Sums over the free (edge) axis; masking already applied on
            # the host (masked-out edges: t=0 -> sin(b) corrected on host;
            # rels/linear rows zeroed).
            nc.vector.tensor_reduce(
                out=r_sum[:], in_=rels_T, axis=mybir.AxisListType.X,
                op=ALU.add,
            )
            nc.vector.tensor_reduce(
                out=te_sum[:], in_=te_t[:], axis=mybir.AxisListType.X,
                op=ALU.add,
            )

            # All contributions accumulate into one single-partition PSUM
            # row (out[0,h] = sum_p lhsT[p,0] * rhs[p,h]) so the final HBM
            # write is a one-descriptor DMA, ordered by operand readiness.
            nc.tensor.matmul(
                out=ps_row[:], lhsT=r_sum[:], rhs=wrv, start=True, stop=False
            )
            nc.tensor.matmul(
                out=ps_row[:], lhsT=vw[:, 0:1], rhs=gath[0][:],
                start=False, stop=False,
            )
            nc.tensor.matmul(
                out=ps_row[:], lhsT=te_sum[:], rhs=wtv, start=False, stop=True
            )
            nc.vector.tensor_copy(out=orow_t[:], in_=ps_row[:])
            nc.sync.dma_start(
                out=out[:].rearrange("(o f) -> o f", o=1), in_=orow_t[:]
            )

    nc.compile()
    return nc


def _prep_in_maps(v_, neighbors, mask, times, rels, w0, b0, w, b, Wt, We):
    PI = float(np.pi)
    v_np = np.ascontiguousarray(np.asarray(v_, dtype=np.float32))
    nb = np.asarray(neighbors).astype(np.int64).ravel()
    m = np.asarray(mask).astype(bool).ravel()
    t = np.asarray(times, dtype=np.float32).ravel()
    rels_np = np.asarray(rels, dtype=np.float32)

    Wtv = np.ascontiguousarray(
        np.asarray(Wt, dtype=np.float32)[:, 2 * HIDDEN : 3 * HIDDEN]
    )
    Wrv = np.ascontiguousarray(
        np.asarray(We, dtype=np.float32)[:, 2 * HIDDEN : 3 * HIDDEN]
    )
    w0f = np.float32(np.asarray(w0))
    b0f = np.float32(np.asarray(b0))
    wf = np.empty(T_DIM, np.float32)
    wf[0] = w0f  # row 0 is a junk sin channel (W_tv row 0 zeroed)
    wf[1:] = np.asarray(w, dtype=np.float32).ravel()
    bf = np.empty(T_DIM, np.float32)
    bf[0] = b0f
    bf[1:] = np.asarray(b, dtype=np.float32).ravel()

    owner = nb // ROWS
    local = (nb - owner * ROWS).astype(np.int32)

    # Host-side masking: masked-out edges get t=0 (each sin channel then
    # contributes exactly sin(b_j); subtracted once below) and zeroed
    # rels/linear rows (zero contribution, no correction needed).
    t_m = np.where(m, t, np.float32(0.0))
    n_zeroed = int((~m).sum())

    in_maps = []
    spill = np.zeros(HIDDEN, np.float64)
    # sin(b) pad correction for the zeroed-time edges (channels 1..63;
    # channel 0 is the host-computed linear row, already masked).
    spill -= n_zeroed * (
        np.sin(bf[1:].astype(np.float64)) @ Wtv[1:].astype(np.float64)
    )
    for c in range(NCORES):
        sel = np.nonzero(m & (owner == c))[0]
        if len(sel) > CAP:  # statistical overflow -> host residual
            over = sel[CAP:]
            sel = sel[:CAP]
            spill += v_np[nb[over]].sum(axis=0, dtype=np.float64)
        n = len(sel)
        idx = np.zeros(CAP, np.int32)
        wts = np.zeros(CAP, np.float32)
        idx[:n] = local[sel]
        wts[:n] = 1.0
        # slot s <-> (chunk j = s // P, partition p = s % P)
        idx_pc = np.ascontiguousarray(idx.reshape(NCHV, P).T)
        wts_pc = np.ascontiguousarray(wts.reshape(NCHV, P).T)

        blk = slice(EB * c, EB * (c + 1))
        mblk = m[blk]
        pkA = np.zeros((T_DIM, FA), np.float32)
        pkA[:, 0:EB] = t_m[blk][None, :]
        pkA[:, EB] = wf
        pkA[:, EB + 1] = bf
        pkA[0:R_DIM, EB + 2 : EB + 2 + EB] = rels_np[blk].T * mblk[None, :]
        pkA[R_DIM, EB + 2 : EB + 2 + EB] = np.where(
            mblk, w0f * t[blk] + b0f, np.float32(0.0)
        )
        pk3 = np.zeros((P, F3), np.float32)
        pk3[0:T_DIM, 0:HIDDEN] = Wtv
        pk3[0, 0:HIDDEN] = 0.0  # linear channel moved to the rels row
        pk3[0:R_DIM, HIDDEN : 2 * HIDDEN] = Wrv
        pk3[R_DIM, HIDDEN : 2 * HIDDEN] = Wtv[0]
        pk3[:, 2 * HIDDEN : 2 * HIDDEN + NCHV] = wts_pc

        in_maps.append(
            {
                "v_shard": v_np[c * ROWS : (c + 1) * ROWS],
                "idx_pc": idx_pc,
                "pkA": pkA,
                "pk3": pk3,
            }
        )
    return in_maps, spill


def kernel(
    k_,
    q_,
    v_,
    neighbors,
    nid,
    mask,
    start_t,
    times,
    rels,
    t2v_w0,
    t2v_b0,
    t2v_w,
    t2v_b,
    time_kqv_w,
    edge_kqv_w,
):
    from concourse.bass_utils import run_bass_kernel_spmd

    nc = _CACHE.get("nc")
    if nc is None:
        nc = _build_program()
        _CACHE["nc"] = nc

    in_maps, spill = _prep_in_maps(
        v_, neighbors, mask, times, rels, t2v_w0, t2v_b0, t2v_w, t2v_b,
        time_kqv_w, edge_kqv_w,
    )
    res = run_bass_kernel_spmd(nc, in_maps, list(range(NCORES)))
    total = np.sum(
        np.stack([r["out"] for r in res.results]).astype(np.float64), axis=0
    )
    return (total + spill).astype(np.float32).reshape(1, HIDDEN)


# revision 18
# speedup vs baseline: 2.9523x; 1.0058x over previous
"""Bass/Trainium2 SPMD kernel for nn_JittableSelfAttention_Rels.

The reference's softmax is over the singleton query dim, so attention
weights are identically 1 and the output reduces to

    out[1,128] = sum_{e: mask[e]} ( v_[neighbors[e]]
                                    + t2v(times[e]) @ W_tv
                                    + rels[e] @ W_rv )

where W_tv / W_rv are the v-thirds of time_kqv_w / edge_kqv_w.

Sharding: v_ is split row-wise across 8 cores. The host routes each
masked edge to the core owning its neighbor row (<=256 slots per core,
11-sigma headroom; overflow edges are summed on the host, which in
practice never triggers). Each core gathers its routed rows with two
128-row indirect DMAs and reduces them with transposed matmuls into a
single-partition [1,128] PSUM row (single-partition output keeps the
final HBM write on one DMA engine, avoiding the 16-way semaphore
trickle). The Time2Vec/rels contributions are position-sharded: core c
handles edges [256c, 256c+256) with T_DIM on partitions, so the whole
sin argument is one fused tensor_scalar, the +-pi range reduction is a
single mod op (arguments pre-biased by +9*pi on the host so trunc- and
floor-mod agree), and the masked sums are fused multiply+reduce ops.
The host adds the 8 partial [128] outputs.
"""

import sys

import numpy as np

if "/opt/trn_rl_repo" not in sys.path:
    sys.path.insert(0, "/opt/trn_rl_repo")

N_NODES = 1_000_000
E = 2048
HIDDEN = 128
P = 128
NCORES = 8
ROWS = N_NODES // NCORES  # 125000
T_DIM = 64
R_DIM = 32
EB = E // NCORES  # 256 edges per core for the t2v/rels block
NCHV = 1          # gather chunks per core
CAP = NCHV * P    # routed v-row slots per core; statistical overflow
                  # (expected ~35 rows across all cores) is summed on the
                  # host via the spill path, which also guarantees
                  # correctness for any input distribution.

# Masked-out edges are neutralized on the HOST: their times are set to 0
# (so each sin channel contributes exactly sin(b_j), corrected once
# globally on the host) and their rels/linear rows are zeroed. This
# removes both mask-multiply ops from the device.
#
# Parameters ride in two packed tensors plus the index table, split by
# when each block is first needed on device. The linear Time2Vec channel
# rides as row 32 of the rels block (its values w0*t+b0 are a host-side
# affine of the shipped times), so every engine op can use partition
# ranges starting at 0 (the BIR verifier rejects odd partition offsets).
# pkA [64, EB+2+EB]: cols 0..255 times (replicated x64, masked-out -> 0),
#                    col 256 w, col 257 b,
#                    cols 258..513 rels transposed [32 rows] + linear
#                    row 32 (masked-out edges zeroed)
# pk3 [128, HIDDEN+NCHV]: cols 0..127 = W128, the stacked projection
#                    matrix: rows 0..63 W_tv (row 0 zeroed), rows 64..95
#                    W_rv, row 96 W_tv's true row 0 (linear channel);
#                    col 128 routed-slot weights (all 128 rows).
# The projection contracts s128 = [te_sum; r_sum] (te channels on
# partitions 0..63 via the Sin activation's accum_out, rels sums on
# partitions 64..127 via the rels reduce) in ONE K=128 matmul.
FA = 2 * EB + 2
F3 = HIDDEN + NCHV

_CACHE = {}


def _build_program():
    import concourse.bass as bass
    import concourse.tile as tile
    from concourse import bacc, mybir

    f32 = mybir.dt.float32
    i32 = mybir.dt.int32
    PI = float(np.pi)
    ALU = mybir.AluOpType

    nc = bacc.Bacc()
    v_shard = nc.declare_dram_parameter("v_shard", [ROWS, HIDDEN], f32, isOutput=False)
    idx_pc = nc.declare_dram_parameter("idx_pc", [P, NCHV], i32, isOutput=False)
    pkA = nc.declare_dram_parameter("pkA", [T_DIM, FA], f32, isOutput=False)
    pk3 = nc.declare_dram_parameter("pk3", [P, F3], f32, isOutput=False)
    out = nc.declare_dram_parameter("out", [HIDDEN], f32, isOutput=True)

    with tile.TileContext(nc) as tc:
        with (
            tc.tile_pool(name="sb", bufs=1) as pool,
            tc.tile_pool(name="ps", bufs=1, space="PSUM") as psum,
        ):
            idx_t = pool.tile([P, NCHV], i32)
            pkA_t = pool.tile([T_DIM, FA], f32)
            pk3_t = pool.tile([P, F3], f32)
            gath = [
                pool.tile([P, HIDDEN], f32, tag=f"g{j}", name=f"gath{j}")
                for j in range(NCHV)
            ]
            te_t = pool.tile([T_DIM, EB], f32)   # arg, then sin(arg)
            u_t = pool.tile([T_DIM, EB], f32)    # arg/2pi + magic
            qm_t = pool.tile([T_DIM, EB], f32)   # -2pi * round(arg/2pi)
            r_t = pool.tile([T_DIM, EB], f32)    # range-reduced argument
            s128 = pool.tile([P, 1], f32)  # [te_sum; r_sum] stacked
            orow_t = pool.tile([1, HIDDEN], f32)
            ps_row = psum.tile([1, HIDDEN], f32, space="PSUM")

            # Index load first so the gathers start as early as possible.
            nc.sync.dma_start(out=idx_t[:], in_=idx_pc[:])
            for j in range(NCHV):
                nc.gpsimd.indirect_dma_start(
                    out=gath[j][:],
                    out_offset=None,
                    in_=v_shard[:],
                    in_offset=bass.IndirectOffsetOnAxis(
                        ap=idx_t[:, j : j + 1], axis=0
                    ),
                )
            nc.sync.dma_start(out=pkA_t[:], in_=pkA[:])
            nc.sync.dma_start(out=pk3_t[:], in_=pk3[:])

            tm_bc = pkA_t[:, 0:EB]
            w_col = pkA_t[:, EB : EB + 1]
            b_col = pkA_t[:, EB + 1 : EB + 2]
            rels_T = pkA_t[:, EB + 2 : EB + 2 + EB]
            w128 = pk3_t[:, 0:HIDDEN]
            vw = pk3_t[:, HIDDEN : HIDDEN + NCHV]

            # arg[j,e] = t[e]*w[j] + b[j]; range-reduce to [-pi, pi] for
            # the Sin table via the fp32 magic-number round:
            # r = arg - 2pi * round(arg/2pi). Row 0's sin is junk (the true
            # linear channel rides in the rels block; W_tv row 0 is zeroed).
            MAGIC = 12582912.0  # 1.5 * 2**23
            nc.vector.tensor_scalar(
                out=te_t[:], in0=tm_bc, scalar1=w_col, scalar2=b_col,
                op0=ALU.mult, op1=ALU.add,
            )
            nc.vector.tensor_scalar(
                out=u_t[:], in0=te_t[:], scalar1=1.0 / (2.0 * PI),
                scalar2=MAGIC, op0=ALU.mult, op1=ALU.add,
            )
            nc.vector.tensor_scalar(
                out=qm_t[:], in0=u_t[:], scalar1=MAGIC,
                scalar2=-2.0 * PI, op0=ALU.subtract, op1=ALU.mult,
            )
            nc.vector.tensor_tensor(
                out=r_t[:], in0=te_t[:], in1=qm_t[:], op=ALU.add
            )
            # The Sin's accum_out IS the te edge-sum (masking applied on
            # the host: masked-out edges have t=0 -> sin(b), corrected on
            # the host); the rels sum reduces into the lower half of s128.
            nc.scalar.activation(
                out=te_t[:], in_=r_t[:],
                func=mybir.ActivationFunctionType.Sin,
                accum_out=s128[0:T_DIM, :],
            )
            nc.vector.tensor_reduce(
                out=s128[T_DIM:P, :], in_=rels_T, axis=mybir.AxisListType.X,
                op=ALU.add,
            )

            # All contributions accumulate into one single-partition PSUM
            # row (out[0,h] = sum_p lhsT[p,0] * rhs[p,h]) so the final HBM
            # write is a one-descriptor DMA, ordered by operand readiness.
            nc.tensor.matmul(
                out=ps_row[:], lhsT=vw[:, 0:1], rhs=gath[0][:],
                start=True, stop=False,
            )
            nc.tensor.matmul(
                out=ps_row[:], lhsT=s128[:], rhs=w128, start=False, stop=True
            )
            nc.vector.tensor_copy(out=orow_t[:], in_=ps_row[:])
            nc.sync.dma_start(
                out=out[:].rearrange("(o f) -> o f", o=1), in_=orow_t[:]
            )

    nc.compile()
    return nc


def _prep_in_maps(v_, neighbors, mask, times, rels, w0, b0, w, b, Wt, We):
    PI = float(np.pi)
    v_np = np.ascontiguousarray(np.asarray(v_, dtype=np.float32))
    nb = np.asarray(neighbors).astype(np.int64).ravel()
    m = np.asarray(mask).astype(bool).ravel()
    t = np.asarray(times, dtype=np.float32).ravel()
    rels_np = np.asarray(rels, dtype=np.float32)

    Wtv = np.ascontiguousarray(
        np.asarray(Wt, dtype=np.float32)[:, 2 * HIDDEN : 3 * HIDDEN]
    )
    Wrv = np.ascontiguousarray(
        np.asarray(We, dtype=np.float32)[:, 2 * HIDDEN : 3 * HIDDEN]
    )
    w0f = np.float32(np.asarray(w0))
    b0f = np.float32(np.asarray(b0))
    wf = np.empty(T_DIM, np.float32)
    wf[0] = w0f  # row 0 is a junk sin channel (W_tv row 0 zeroed)
    wf[1:] = np.asarray(w, dtype=np.float32).ravel()
    bf = np.empty(T_DIM, np.float32)
    bf[0] = b0f
    bf[1:] = np.asarray(b, dtype=np.float32).ravel()

    owner = nb // ROWS
    local = (nb - owner * ROWS).astype(np.int32)

    # Host-side masking: masked-out edges get t=0 (each sin channel then
    # contributes exactly sin(b_j); subtracted once below) and zeroed
    # rels/linear rows (zero contribution, no correction needed).
    t_m = np.where(m, t, np.float32(0.0))
    n_zeroed = int((~m).sum())

    in_maps = []
    spill = np.zeros(HIDDEN, np.float64)
    # sin(b) pad correction for the zeroed-time edges (channels 1..63;
    # channel 0 is the host-computed linear row, already masked).
    spill -= n_zeroed * (
        np.sin(bf[1:].astype(np.float64)) @ Wtv[1:].astype(np.float64)
    )
    for c in range(NCORES):
        sel = np.nonzero(m & (owner == c))[0]
        if len(sel) > CAP:  # statistical overflow -> host residual
            over = sel[CAP:]
            sel = sel[:CAP]
            spill += v_np[nb[over]].sum(axis=0, dtype=np.float64)
        n = len(sel)
        idx = np.zeros(CAP, np.int32)
        wts = np.zeros(CAP, np.float32)
        idx[:n] = local[sel]
        wts[:n] = 1.0
        # slot s <-> (chunk j = s // P, partition p = s % P)
        idx_pc = np.ascontiguousarray(idx.reshape(NCHV, P).T)
        wts_pc = np.ascontiguousarray(wts.reshape(NCHV, P).T)

        blk = slice(EB * c, EB * (c + 1))
        mblk = m[blk]
        pkA = np.zeros((T_DIM, FA), np.float32)
        pkA[:, 0:EB] = t_m[blk][None, :]
        pkA[:, EB] = wf
        pkA[:, EB + 1] = bf
        pkA[0:R_DIM, EB + 2 : EB + 2 + EB] = rels_np[blk].T * mblk[None, :]
        pkA[R_DIM, EB + 2 : EB + 2 + EB] = np.where(
            mblk, w0f * t[blk] + b0f, np.float32(0.0)
        )
        pk3 = np.zeros((P, F3), np.float32)
        pk3[0:T_DIM, 0:HIDDEN] = Wtv
        pk3[0, 0:HIDDEN] = 0.0  # linear channel moved to the rels row
        pk3[T_DIM : T_DIM + R_DIM, 0:HIDDEN] = Wrv
        pk3[T_DIM + R_DIM, 0:HIDDEN] = Wtv[0]
        pk3[:, HIDDEN : HIDDEN + NCHV] = wts_pc

        in_maps.append(
            {
                "v_shard": v_np[c * ROWS : (c + 1) * ROWS],
                "idx_pc": idx_pc,
                "pkA": pkA,
                "pk3": pk3,
            }
        )
    return in_maps, spill


def kernel(
    k_,
    q_,
    v_,
    neighbors,
    nid,
    mask,
    start_t,
    times,
    rels,
    t2v_w0,
    t2v_b0,
    t2v_w,
    t2v_b,
    time_kqv_w,
    edge_kqv_w,
):
    from concourse.bass_utils import run_bass_kernel_spmd

    nc = _CACHE.get("nc")
    if nc is None:
        nc = _build_program()
        _CACHE["nc"] = nc

    in_maps, spill = _prep_in_maps(
        v_, neighbors, mask, times, rels, t2v_w0, t2v_b0, t2v_w, t2v_b,
        time_kqv_w, edge_kqv_w,
    )
    res = run_bass_kernel_spmd(nc, in_maps, list(range(NCORES)))
    total = np.sum(
        np.stack([r["out"] for r in res.results]).astype(np.float64), axis=0
    )
    return (total + spill).astype(np.float32).reshape(1, HIDDEN)


# revision 20
# speedup vs baseline: 3.0012x; 1.0166x over previous
"""Bass/Trainium2 SPMD kernel for nn_JittableSelfAttention_Rels.

The reference's softmax is over the singleton query dim, so attention
weights are identically 1 and the output reduces to

    out[1,128] = sum_{e: mask[e]} ( v_[neighbors[e]]
                                    + t2v(times[e]) @ W_tv
                                    + rels[e] @ W_rv )

where W_tv / W_rv are the v-thirds of time_kqv_w / edge_kqv_w.

Sharding: v_ is split row-wise across 8 cores. The host routes each
masked edge to the core owning its neighbor row (<=256 slots per core,
11-sigma headroom; overflow edges are summed on the host, which in
practice never triggers). Each core gathers its routed rows with two
128-row indirect DMAs and reduces them with transposed matmuls into a
single-partition [1,128] PSUM row (single-partition output keeps the
final HBM write on one DMA engine, avoiding the 16-way semaphore
trickle). The Time2Vec/rels contributions are position-sharded: core c
handles edges [256c, 256c+256) with T_DIM on partitions, so the whole
sin argument is one fused tensor_scalar, the +-pi range reduction is a
single mod op (arguments pre-biased by +9*pi on the host so trunc- and
floor-mod agree), and the masked sums are fused multiply+reduce ops.
The host adds the 8 partial [128] outputs.
"""

import sys

import numpy as np

if "/opt/trn_rl_repo" not in sys.path:
    sys.path.insert(0, "/opt/trn_rl_repo")

N_NODES = 1_000_000
E = 2048
HIDDEN = 128
P = 128
NCORES = 8
ROWS = N_NODES // NCORES  # 125000
T_DIM = 64
R_DIM = 32
EB = E // NCORES  # 256 edges per core for the t2v/rels block
NCHV = 1          # gather chunks per core
CAP = NCHV * P    # routed v-row slots per core; statistical overflow
                  # (expected ~35 rows across all cores) is summed on the
                  # host via the spill path, which also guarantees
                  # correctness for any input distribution.

# Masked-out edges are neutralized on the HOST: their times are set to 0
# (so each sin channel contributes exactly sin(b_j), corrected once
# globally on the host) and their rels/linear rows are zeroed. This
# removes both mask-multiply ops from the device.
#
# Parameters ride in two packed tensors plus the index table, split by
# when each block is first needed on device. The linear Time2Vec channel
# rides as row 32 of the rels block (its values w0*t+b0 are a host-side
# affine of the shipped times), so every engine op can use partition
# ranges starting at 0 (the BIR verifier rejects odd partition offsets).
# pkA [64, EB+2+EB]: cols 0..255 times (replicated x64, masked-out -> 0),
#                    col 256 w, col 257 b,
#                    cols 258..513 rels transposed [32 rows] + linear
#                    row 32 (masked-out edges zeroed)
# pk3 [128, HIDDEN+NCHV]: cols 0..127 = W128, the stacked projection
#                    matrix: rows 0..63 W_tv (row 0 zeroed), rows 64..95
#                    W_rv, row 96 W_tv's true row 0 (linear channel);
#                    col 128 routed-slot weights (all 128 rows).
# The projection contracts s128 = [te_sum; r_sum] (te channels on
# partitions 0..63 via the Sin activation's accum_out, rels sums on
# partitions 64..127 via the rels reduce) in ONE K=128 matmul.
FA = 2 * EB + 2
F3 = HIDDEN + NCHV

_CACHE = {}


def _build_program():
    import concourse.bass as bass
    import concourse.tile as tile
    from concourse import bacc, mybir
    from concourse.tile import add_dep_helper

    f32 = mybir.dt.float32
    i32 = mybir.dt.int32
    PI = float(np.pi)
    ALU = mybir.AluOpType

    nc = bacc.Bacc()
    v_shard = nc.declare_dram_parameter("v_shard", [ROWS, HIDDEN], f32, isOutput=False)
    idx_pc = nc.declare_dram_parameter("idx_pc", [P, NCHV], i32, isOutput=False)
    pkA = nc.declare_dram_parameter("pkA", [T_DIM, FA], f32, isOutput=False)
    pk3 = nc.declare_dram_parameter("pk3", [P, F3], f32, isOutput=False)
    out = nc.declare_dram_parameter("out", [HIDDEN], f32, isOutput=True)

    with tile.TileContext(nc) as tc:
        with (
            tc.tile_pool(name="sb", bufs=1) as pool,
            tc.tile_pool(name="ps", bufs=1, space="PSUM") as psum,
        ):
            idx_t = pool.tile([P, NCHV], i32)
            pkA_t = pool.tile([T_DIM, FA], f32)
            pk3_t = pool.tile([P, F3], f32)
            gath = [
                pool.tile([P, HIDDEN], f32, tag=f"g{j}", name=f"gath{j}")
                for j in range(NCHV)
            ]
            te_t = pool.tile([T_DIM, EB], f32)   # arg, then sin(arg)
            u_t = pool.tile([T_DIM, EB], f32)    # arg/2pi + magic
            qm_t = pool.tile([T_DIM, EB], f32)   # -2pi * round(arg/2pi)
            r_t = pool.tile([T_DIM, EB], f32)    # range-reduced argument
            s128 = pool.tile([P, 1], f32)  # [te_sum; r_sum] stacked
            orow_t = pool.tile([1, HIDDEN], f32)
            ps_row = psum.tile([1, HIDDEN], f32, space="PSUM")

            # Index load first so the gathers start as early as possible.
            nc.sync.dma_start(out=idx_t[:], in_=idx_pc[:])
            for j in range(NCHV):
                nc.gpsimd.indirect_dma_start(
                    out=gath[j][:],
                    out_offset=None,
                    in_=v_shard[:],
                    in_offset=bass.IndirectOffsetOnAxis(
                        ap=idx_t[:, j : j + 1], axis=0
                    ),
                )
            nc.sync.dma_start(out=pkA_t[:], in_=pkA[:])
            nc.sync.dma_start(out=pk3_t[:], in_=pk3[:])

            tm_bc = pkA_t[:, 0:EB]
            w_col = pkA_t[:, EB : EB + 1]
            b_col = pkA_t[:, EB + 1 : EB + 2]
            rels_T = pkA_t[:, EB + 2 : EB + 2 + EB]
            w128 = pk3_t[:, 0:HIDDEN]
            vw = pk3_t[:, HIDDEN : HIDDEN + NCHV]

            # arg[j,e] = t[e]*w[j] + b[j]; range-reduce to [-pi, pi] for
            # the Sin table via the fp32 magic-number round:
            # r = arg - 2pi * round(arg/2pi). Row 0's sin is junk (the true
            # linear channel rides in the rels block; W_tv row 0 is zeroed).
            MAGIC = 12582912.0  # 1.5 * 2**23
            nc.vector.tensor_scalar(
                out=te_t[:], in0=tm_bc, scalar1=w_col, scalar2=b_col,
                op0=ALU.mult, op1=ALU.add,
            )
            nc.vector.tensor_scalar(
                out=u_t[:], in0=te_t[:], scalar1=1.0 / (2.0 * PI),
                scalar2=MAGIC, op0=ALU.mult, op1=ALU.add,
            )
            nc.vector.tensor_scalar(
                out=qm_t[:], in0=u_t[:], scalar1=MAGIC,
                scalar2=-2.0 * PI, op0=ALU.subtract, op1=ALU.mult,
            )
            r_inst = nc.vector.tensor_tensor(
                out=r_t[:], in0=te_t[:], in1=qm_t[:], op=ALU.add
            )
            # The Sin's accum_out IS the te edge-sum (masking applied on
            # the host: masked-out edges have t=0 -> sin(b), corrected on
            # the host); the rels sum reduces into the lower half of s128.
            nc.scalar.activation(
                out=te_t[:], in_=r_t[:],
                func=mybir.ActivationFunctionType.Sin,
                accum_out=s128[0:T_DIM, :],
            )
            red_inst = nc.vector.tensor_reduce(
                out=s128[T_DIM:P, :], in_=rels_T, axis=mybir.AxisListType.X,
                op=ALU.add,
            )
            # Keep the serial sin-argument chain tight on DVE: the rels
            # reduce has slack (it overlaps the Sin on the scalar engine).
            add_dep_helper(
                red_inst.ins, r_inst.ins, sync=False,
                reason="rels reduce after sin-arg chain",
            )

            # All contributions accumulate into one single-partition PSUM
            # row (out[0,h] = sum_p lhsT[p,0] * rhs[p,h]) so the final HBM
            # write is a one-descriptor DMA, ordered by operand readiness.
            nc.tensor.matmul(
                out=ps_row[:], lhsT=vw[:, 0:1], rhs=gath[0][:],
                start=True, stop=False,
            )
            nc.tensor.matmul(
                out=ps_row[:], lhsT=s128[:], rhs=w128, start=False, stop=True
            )
            nc.vector.tensor_copy(out=orow_t[:], in_=ps_row[:])
            nc.sync.dma_start(
                out=out[:].rearrange("(o f) -> o f", o=1), in_=orow_t[:]
            )

    nc.compile()
    return nc


def _prep_in_maps(v_, neighbors, mask, times, rels, w0, b0, w, b, Wt, We):
    PI = float(np.pi)
    v_np = np.ascontiguousarray(np.asarray(v_, dtype=np.float32))
    nb = np.asarray(neighbors).astype(np.int64).ravel()
    m = np.asarray(mask).astype(bool).ravel()
    t = np.asarray(times, dtype=np.float32).ravel()
    rels_np = np.asarray(rels, dtype=np.float32)

    Wtv = np.ascontiguousarray(
        np.asarray(Wt, dtype=np.float32)[:, 2 * HIDDEN : 3 * HIDDEN]
    )
    Wrv = np.ascontiguousarray(
        np.asarray(We, dtype=np.float32)[:, 2 * HIDDEN : 3 * HIDDEN]
    )
    w0f = np.float32(np.asarray(w0))
    b0f = np.float32(np.asarray(b0))
    wf = np.empty(T_DIM, np.float32)
    wf[0] = w0f  # row 0 is a junk sin channel (W_tv row 0 zeroed)
    wf[1:] = np.asarray(w, dtype=np.float32).ravel()
    bf = np.empty(T_DIM, np.float32)
    bf[0] = b0f
    bf[1:] = np.asarray(b, dtype=np.float32).ravel()

    owner = nb // ROWS
    local = (nb - owner * ROWS).astype(np.int32)

    # Host-side masking: masked-out edges get t=0 (each sin channel then
    # contributes exactly sin(b_j); subtracted once below) and zeroed
    # rels/linear rows (zero contribution, no correction needed).
    t_m = np.where(m, t, np.float32(0.0))
    n_zeroed = int((~m).sum())

    in_maps = []
    spill = np.zeros(HIDDEN, np.float64)
    # sin(b) pad correction for the zeroed-time edges (channels 1..63;
    # channel 0 is the host-computed linear row, already masked).
    spill -= n_zeroed * (
        np.sin(bf[1:].astype(np.float64)) @ Wtv[1:].astype(np.float64)
    )
    for c in range(NCORES):
        sel = np.nonzero(m & (owner == c))[0]
        if len(sel) > CAP:  # statistical overflow -> host residual
            over = sel[CAP:]
            sel = sel[:CAP]
            spill += v_np[nb[over]].sum(axis=0, dtype=np.float64)
        n = len(sel)
        idx = np.zeros(CAP, np.int32)
        wts = np.zeros(CAP, np.float32)
        idx[:n] = local[sel]
        wts[:n] = 1.0
        # slot s <-> (chunk j = s // P, partition p = s % P)
        idx_pc = np.ascontiguousarray(idx.reshape(NCHV, P).T)
        wts_pc = np.ascontiguousarray(wts.reshape(NCHV, P).T)

        blk = slice(EB * c, EB * (c + 1))
        mblk = m[blk]
        pkA = np.zeros((T_DIM, FA), np.float32)
        pkA[:, 0:EB] = t_m[blk][None, :]
        pkA[:, EB] = wf
        pkA[:, EB + 1] = bf
        pkA[0:R_DIM, EB + 2 : EB + 2 + EB] = rels_np[blk].T * mblk[None, :]
        pkA[R_DIM, EB + 2 : EB + 2 + EB] = np.where(
            mblk, w0f * t[blk] + b0f, np.float32(0.0)
        )
        pk3 = np.zeros((P, F3), np.float32)
        pk3[0:T_DIM, 0:HIDDEN] = Wtv
        pk3[0, 0:HIDDEN] = 0.0  # linear channel moved to the rels row
        pk3[T_DIM : T_DIM + R_DIM, 0:HIDDEN] = Wrv
        pk3[T_DIM + R_DIM, 0:HIDDEN] = Wtv[0]
        pk3[:, HIDDEN : HIDDEN + NCHV] = wts_pc

        in_maps.append(
            {
                "v_shard": v_np[c * ROWS : (c + 1) * ROWS],
                "idx_pc": idx_pc,
                "pkA": pkA,
                "pk3": pk3,
            }
        )
    return in_maps, spill


def kernel(
    k_,
    q_,
    v_,
    neighbors,
    nid,
    mask,
    start_t,
    times,
    rels,
    t2v_w0,
    t2v_b0,
    t2v_w,
    t2v_b,
    time_kqv_w,
    edge_kqv_w,
):
    from concourse.bass_utils import run_bass_kernel_spmd

    nc = _CACHE.get("nc")
    if nc is None:
        nc = _build_program()
        _CACHE["nc"] = nc

    in_maps, spill = _prep_in_maps(
        v_, neighbors, mask, times, rels, t2v_w0, t2v_b0, t2v_w, t2v_b,
        time_kqv_w, edge_kqv_w,
    )
    res = run_bass_kernel_spmd(nc, in_maps, list(range(NCORES)))
    total = np.sum(
        np.stack([r["out"] for r in res.results]).astype(np.float64), axis=0
    )
    return (total + spill).astype(np.float32).reshape(1, HIDDEN)
